# revision 3
# baseline (speedup 1.0000x reference)
"""Distributed causal multi-head attention for Trainium2 (8 NeuronCores).

Problem: B=2, S=2048, NX=1024, H=16 heads, D=64.
  qkv = x @ w_attn + b_attn ; q,k,v split; causal softmax(q k^T / 8) v ; @ w_proj + b_proj

Sharding: core c -> batch b=c//4 (data parallel), head group g=c%4 (tensor
parallel, 4 heads). Column-split c_attn; after attention four per-head
AllToAlls reshard heads->sequence so each core computes c_proj for its own
512 output rows with the full hidden dim - no cross-core reduction.

Schedule (v2): phase 1 computes only h0/h1's q,k (fi0/fi2) plus most of v;
h2/h3's q,k chains are woven into h0's attention as PE filler. Attention
runs a global 1-block software pipeline: the score matmuls for block
(h,kb+1) are emitted BEFORE the PV of block (h,kb), so ScalarE's exp of
block kb runs concurrently with the next block's score matmuls and the PE
never stalls on exp. All PSUM->SBUF copies during the attention window run
on DVE (tensor_scalar_add / tensor_copy), keeping ScalarE exp-only; the
1/sqrt(D)=1/8 score scale is folded into the q weights host-side (exact in
bf16 - exponent shift). c_proj slices for head h are woven into head h+1's
blocks; all 8 of h2's slices are held back to cover the last A2A's
sync+transfer window, and h3's slices stream per-(st,nn2) output DMAs so
the final HBM writes overlap the remaining matmuls.
"""

import sys

sys.path.insert(0, "/opt/trn_rl_repo")

import numpy as np
import ml_dtypes

BF16 = ml_dtypes.bfloat16

B = 2
S = 2048
NX = 1024
H = 16
D = 64
G = 4            # head groups (tensor-parallel)
HL = H // G      # heads per core = 4
HDW = HL * D     # head-group width = 256
P = 128
SC = 512         # output chunk (A2A granularity)
NQC = S // SC    # 4 chunks
NE = NX // P     # 8 contraction tiles
NKB = S // P     # 16 key blocks
WQ = 1024        # max score-tile width

_COMPILED = None


def _build():
    import concourse.bass as bass  # noqa: F401
    from concourse.bass import ds
    import concourse.mybir as mybir
    import concourse.tile as tile
    from concourse import bacc

    f32 = mybir.dt.float32
    i32 = mybir.dt.int32
    bf16 = mybir.dt.bfloat16
    Identity = mybir.ActivationFunctionType.Identity
    Exp = mybir.ActivationFunctionType.Exp

    nc = bacc.Bacc("TRN2", target_bir_lowering=False, debug=False, num_devices=8)

    x0 = nc.dram_tensor("x0", [NE, P, SC], bf16, kind="ExternalInput")
    xr = nc.dram_tensor("xr", [NE, P, S - SC], bf16, kind="ExternalInput")
    wqkf = nc.dram_tensor("wqkf", [4, P, NX], bf16, kind="ExternalInput")
    wvc = nc.dram_tensor("wvc", [P, NE * HDW], bf16, kind="ExternalInput")
    wpc = nc.dram_tensor("wpc", [P, 2 * HL * NX], bf16, kind="ExternalInput")
    bqk = nc.dram_tensor("bqk", [4, P], f32, kind="ExternalInput")
    bp32 = nc.dram_tensor("bp32", [1, NX], f32, kind="ExternalInput")
    onesb = nc.dram_tensor("onesb", [P, 4], bf16, kind="ExternalInput")
    causb = nc.dram_tensor("causb", [P, P], bf16, kind="ExternalInput")
    slotb = nc.dram_tensor("slotb", [1, 4], i32, kind="ExternalInput")
    out_ext = nc.dram_tensor("out", [SC, NX], f32, kind="ExternalOutput")

    with tile.TileContext(nc) as tc:
        with (
            tc.tile_pool(name="const", bufs=1) as const_pool,
            tc.tile_pool(name="xt", bufs=1) as xt_pool,
            tc.tile_pool(name="w", bufs=1) as w_pool,
            tc.tile_pool(name="qkt", bufs=1) as qkt_pool,
            tc.tile_pool(name="vsb", bufs=1) as v_pool,
            tc.tile_pool(name="lh", bufs=1) as lh_pool,
            tc.tile_pool(name="exp", bufs=12) as exp_pool,
            tc.tile_pool(name="osb", bufs=1) as osb_pool,
            tc.tile_pool(name="small", bufs=3) as small_pool,
            tc.tile_pool(name="wide", bufs=2, space="PSUM") as wide_ps,
            tc.tile_pool(name="atps", bufs=4, space="PSUM") as at_ps_pool,
            tc.tile_pool(name="dram", bufs=1, space="DRAM") as dram_pool,
        ):
            # ---- constants + the batch slot-base register (4*b) ----
            slot_sb = const_pool.tile([1, 4], i32, name="slot_sb")
            nc.sync.dma_start(slot_sb[:], slotb[:])
            sreg = nc.sync.alloc_register("slotreg")
            nc.sync.reg_load(sreg, slot_sb[0:1, 0:1])
            rv = nc.sync.snap(sreg, donate=True, min_val=0, max_val=4)

            bqk_sb = const_pool.tile([P, 4], f32, name="bqk_sb")
            for fi in range(4):
                nc.sync.dma_start(bqk_sb[:, fi : fi + 1], bqk[fi : fi + 1, :])
            bp_sb = const_pool.tile([1, NX], f32, name="bp_sb")
            nc.sync.dma_start(bp_sb[:], bp32[:])
            czb_sb = const_pool.tile([P, P], bf16, name="czb_sb")
            nc.sync.dma_start(czb_sb[:], causb[:])

            # ---- weight + x loads: first-needed first ----
            wqk_sb = {}
            for fi in range(4):
                wqk_sb[fi] = w_pool.tile([P, NX], bf16, name=f"wqk_sb{fi}")
            nc.sync.dma_start(wqk_sb[0][:], wqkf[0])
            nc.sync.dma_start(wqk_sb[2][:], wqkf[2])
            xt0_sb = {}
            wv_sb = {}
            for e in range(NE):
                t = xt_pool.tile([P, SC], bf16, name=f"xt0_{e}")
                nc.sync.dma_start(t[:], x0[e])
                xt0_sb[e] = t
                tv = w_pool.tile([P, HDW], bf16, name=f"wv_sb{e}")
                nc.sync.dma_start(tv[:], wvc[:, e * HDW : (e + 1) * HDW])
                wv_sb[e] = tv
            xtr_sb = {}
            for e in range(NE):
                t = xt_pool.tile([P, S - SC], bf16, name=f"xtr_{e}")
                nc.sync.dma_start(t[:], xr[e])
                xtr_sb[e] = t
            nc.sync.dma_start(wqk_sb[1][:], wqkf[1])
            nc.sync.dma_start(wqk_sb[3][:], wqkf[3])
            wp_sb = w_pool.tile([P, 2 * HL * NX], bf16, name="wp_sb")
            nc.sync.dma_start(wp_sb[:], wpc[:])

            def xt_slice(e, c0, w):
                if c0 < SC:
                    return xt0_sb[e][:, c0 : c0 + w]
                return xtr_sb[e][:, c0 - SC : c0 - SC + w]

            # ---- persistent q/k/v SBUF state ----
            qkt_sb = {}
            for fi in range(2):
                qkt_sb[fi] = qkt_pool.tile(
                    [P, S], bf16, name=f"qkt{fi}", tag=f"qktw{fi}"
                )
            # per-head kT with the other head's rows zeroed: score matmuls
            # run at K=128 (zeros annihilate the foreign q rows), keeping
            # switching activity low for the HAM power governor
            ktz_sb = {}
            for h in range(HL):
                ktz_sb[h] = qkt_pool.tile([P, S], bf16, name=f"ktz{h}", tag=f"ktz{h}")
                nc.gpsimd.memset(ktz_sb[h][:], 0.0)
            v_sb = {}

            # ---- chain emitters (qk / v); copies on ACT in phase 1,
            # DVE during the attention window so ScalarE stays exp-only ----
            def emit_qk_chain(fi, sc, on_act):
                ps = at_ps_pool.tile(
                    [P, SC], f32, tag="atps", name=f"qk_ps{fi}_{sc}"
                )
                for e in range(NE):
                    nc.tensor.matmul(
                        ps[:],
                        wqk_sb[fi][:, e * P : (e + 1) * P],
                        xt_slice(e, sc * SC, SC),
                        start=(e == 0),
                        stop=(e == NE - 1),
                    )
                cols = slice(sc * SC, (sc + 1) * SC)
                if fi < 2:
                    if on_act:
                        nc.scalar.activation(
                            qkt_sb[fi][:, cols], ps[:], Identity,
                            bias=bqk_sb[:, fi : fi + 1],
                        )
                    else:
                        nc.vector.tensor_scalar_add(
                            qkt_sb[fi][:, cols], ps[:], bqk_sb[:, fi : fi + 1]
                        )
                else:
                    for hh in range(2):
                        h = 2 * (fi - 2) + hh
                        r0 = 64 * hh
                        if on_act:
                            nc.scalar.activation(
                                ktz_sb[h][r0 : r0 + D, cols],
                                ps[r0 : r0 + D, :],
                                Identity,
                                bias=bqk_sb[r0 : r0 + D, fi : fi + 1],
                            )
                        else:
                            nc.vector.tensor_scalar_add(
                                ktz_sb[h][r0 : r0 + D, cols],
                                ps[r0 : r0 + D, :],
                                bqk_sb[r0 : r0 + D, fi : fi + 1],
                            )

            def emit_v_chain(si, on_act):
                sc, j = divmod(si, 4)
                psv = wide_ps.tile([P, HDW], f32, tag="wide", name=f"v_ps{si}")
                for e in range(NE):
                    nc.tensor.matmul(
                        psv[:],
                        xt_slice(e, sc * SC + j * P, P),
                        wv_sb[e][:],
                        start=(e == 0),
                        stop=(e == NE - 1),
                    )
                # per-head 128-wide slots: [v(64) | ones(1) | zeros(63)]
                vt = v_pool.tile([P, HL * P], bf16, name=f"v{si}")
                nc.gpsimd.memset(vt[:], 0.0)
                nc.sync.dma_start(
                    vt[:].rearrange("p (h u) -> p h u", h=HL)[:, :, D : D + 1],
                    onesb[:],
                )
                dst = vt[:].rearrange("p (h u) -> p h u", h=HL)[:, :, 0:D]
                src = psv[:].rearrange("p (h u) -> p h u", h=HL)
                if on_act:
                    nc.scalar.activation(dst, src, Identity)
                else:
                    nc.vector.tensor_copy(dst, src)
                v_sb[si] = vt

            # ---- phase 1a: fi0/fi2 q,k + v for token blocks 0..11;
            # sc0 first (x0-only work) so the xtr DMAs can land ----
            emit_qk_chain(0, 0, True)
            emit_v_chain(0, True)
            emit_qk_chain(2, 0, True)
            emit_v_chain(1, True)
            emit_v_chain(2, True)
            emit_v_chain(3, True)
            emit_qk_chain(0, 1, True)
            emit_v_chain(4, True)
            emit_qk_chain(2, 1, True)
            emit_v_chain(5, True)
            emit_v_chain(6, True)
            emit_v_chain(7, True)
            emit_qk_chain(0, 2, True)
            emit_v_chain(8, True)
            emit_qk_chain(2, 2, True)
            emit_v_chain(9, True)
            emit_v_chain(10, True)
            emit_v_chain(11, True)
            emit_qk_chain(0, 3, True)
            emit_qk_chain(2, 3, True)

            # ---- A2A buffers ----
            a2a_in = {}
            a2a_out = {}
            lhsrc = {}
            for h in range(HL):
                a2a_in[h] = dram_pool.tile([8, D, SC], bf16, name=f"a2a_in{h}")
                a2a_out[h] = dram_pool.tile([8, D, SC], bf16, name=f"a2a_out{h}")
                lhsrc[h] = dram_pool.tile([4, D, SC], bf16, name=f"lhsrc{h}")

            def emit_tail(h, qc, at_ps):
                # softmax denominator comes from the ones-column of v; the
                # v-bias is folded into bp on the host
                dn32 = small_pool.tile([1, SC], f32, tag="dn32", name=f"dn32{qc}_{h}")
                nc.vector.tensor_copy(dn32[:], at_ps[D : D + 1, :])
                rc32 = small_pool.tile([1, SC], f32, tag="rc32", name=f"rc32{qc}_{h}")
                nc.vector.reciprocal_approx_fast(rc32[:], dn32[:])
                rb = small_pool.tile([D, SC], f32, tag="rbsb", name=f"rbsb{qc}_{h}")
                nc.gpsimd.partition_broadcast(rb[:], rc32[:])
                ath = small_pool.tile(
                    [D, SC], bf16, tag="ath", bufs=3, name=f"ath{qc}_{h}"
                )
                nc.vector.tensor_mul(ath[:], at_ps[0:D, :], rb[:])
                nc.sync.dma_start(a2a_in[h][qc, :, :], ath[:])
                nc.sync.dma_start(a2a_in[h][qc + 4, :, :], ath[:])

            def emit_score_exp(h, kb):
                # score tiles (transposed [k, q]) + exp for key block kb
                fi_q = h // 2
                q0 = P * kb
                width = S - q0
                out = []
                for s2 in range((width + WQ - 1) // WQ):
                    w0 = q0 + s2 * WQ
                    ww = min(WQ, S - w0)
                    scp = wide_ps.tile(
                        [P, WQ], f32, tag="wide", name=f"sc{h}_{kb}_{s2}"
                    )
                    for m0 in range(0, ww, SC):
                        mw = min(SC, ww - m0)
                        nc.tensor.matmul(
                            scp[:, m0 : m0 + mw],
                            ktz_sb[h][:, q0 : q0 + P],
                            qkt_sb[fi_q][:, w0 + m0 : w0 + m0 + mw],
                            start=True,
                            stop=True,
                        )
                    ex = exp_pool.tile(
                        [P, WQ], bf16, tag="exp", name=f"ex{h}_{kb}_{s2}"
                    )
                    nc.scalar.activation(ex[:, 0:ww], scp[:, 0:ww], Exp)
                    if s2 == 0:
                        # diagonal block: zero the non-causal upper triangle
                        nc.vector.tensor_mul(ex[:, 0:P], ex[:, 0:P], czb_sb[:])
                    out.append((ex, w0, ww))
                return out

            def emit_pv(h, kb, exl, at_ps):
                for ex, w0, ww in exl:
                    qc_lo = w0 // SC
                    qc_hi = (w0 + ww - 1) // SC
                    for qc in range(qc_lo, qc_hi + 1):
                        a0 = max(w0, qc * SC)
                        a1 = min(w0 + ww, (qc + 1) * SC)
                        nc.tensor.matmul(
                            at_ps[qc][:, a0 - qc * SC : a1 - qc * SC],
                            v_sb[kb][:, h * P : (h + 1) * P],
                            ex[:, a0 - w0 : a1 - w0],
                            start=(kb == 0),
                            stop=(kb == 4 * qc + 3),
                        )

            # ---- c_proj state + emitters ----
            osb = {}
            for st in range(4):
                osb[st] = osb_pool.tile([P, NX], f32, name=f"osb{st}")
                nc.gpsimd.partition_broadcast(osb[st][:], bp_sb[:])
            lh = {}

            def emit_lh_loads(h):
                for tp in range(2):
                    t = lh_pool.tile([P, SC], bf16, name=f"lh{h}_{tp}")
                    # two half-tile loads land on different DMA queues
                    nc.sync.dma_start(t[0:D, :], lhsrc[h][2 * tp])
                    nc.sync.dma_start(t[D:P, :], lhsrc[h][2 * tp + 1])
                    lh[h, tp] = t

            def emit_proj_group(h, st, nn2, dma_out=False):
                pp = wide_ps.tile([P, SC], f32, tag="wide", name=f"pj{h}_{st}_{nn2}")
                for tp in range(2):
                    wcol = (2 * h + tp) * NX + nn2 * SC
                    nc.tensor.matmul(
                        pp[:],
                        lh[h, tp][:, st * P : (st + 1) * P],
                        wp_sb[:, wcol : wcol + SC],
                        start=(tp == 0),
                        stop=(tp == 1),
                    )
                dst = osb[st][:, nn2 * SC : (nn2 + 1) * SC]
                nc.vector.tensor_add(dst, dst, pp[:])
                if dma_out:
                    nc.sync.dma_start(
                        out_ext[st * P : (st + 1) * P, nn2 * SC : (nn2 + 1) * SC],
                        dst,
                    )

            # ---- dummy A2A: resyncs core drift before the real A2As and
            # absorbs the collective stream's one-time setup cost ----
            a2ad_in = dram_pool.tile([8, 1, P], bf16, name="a2ad_in")
            a2ad_out = dram_pool.tile([8, 1, P], bf16, name="a2ad_out")
            for s in range(8):
                nc.sync.dma_start(a2ad_in[s], v_sb[11][0:1, 0:P])
            nc.gpsimd.collective_compute(
                "AllToAll",
                mybir.AluOpType.bypass,
                ins=[a2ad_in[:].opt()],
                outs=[a2ad_out[:].opt()],
                replica_groups=[list(range(8))],
            )

            # ---- fill schedule: work woven between score(kb+1) and pv(kb).
            # h0: remaining v chains + h2/h3's q,k chains (copies on DVE);
            # h1: h0's proj slices; h2: h1's; h3: lean (h2's slices are the
            # reserve that covers the final A2A window).
            fills = {}
            for j in range(4):
                fills[0, j] = ("v", 12 + j)
            for sc in range(4):
                fills[0, 4 + sc] = ("qk", 1, sc)
            for sc in range(4):
                fills[0, 8 + sc] = ("qk", 3, sc)
            for j in range(8):
                fills[1, 4 + j] = ("proj", 0, j // 2, j % 2)
            for j in range(8):
                fills[2, 4 + j] = ("proj", 1, j // 2, j % 2)

            def emit_fill(f):
                if f[0] == "v":
                    emit_v_chain(f[1], on_act=False)
                elif f[0] == "qk":
                    emit_qk_chain(f[1], f[2], on_act=False)
                else:
                    emit_proj_group(f[1], f[2], f[3])

            # ---- attention: global 1-block software pipeline ----
            blocks = [(h, kb) for h in range(HL) for kb in range(NKB)]
            at_ps_all = {}
            for h in range(HL):
                at_ps_all[h] = {}
            exl_next = emit_score_exp(0, 0)
            for i, (h, kb) in enumerate(blocks):
                if kb == 0:
                    for qc in range(NQC):
                        at_ps_all[h][qc] = at_ps_pool.tile(
                            [P, SC], f32, tag="atps", name=f"at_ps{qc}_{h}"
                        )
                exl = exl_next
                if i + 1 < len(blocks):
                    exl_next = emit_score_exp(*blocks[i + 1])
                f = fills.get((h, kb))
                if f is not None:
                    emit_fill(f)
                emit_pv(h, kb, exl, at_ps_all[h])
                for qc in range(NQC):
                    if kb == 4 * qc + 3:
                        emit_tail(h, qc, at_ps_all[h][qc])
                if kb == NKB - 1:
                    nc.gpsimd.collective_compute(
                        "AllToAll",
                        mybir.AluOpType.bypass,
                        ins=[a2a_in[h][:].opt()],
                        outs=[a2a_out[h][:].opt()],
                        replica_groups=[list(range(8))],
                    )
                    # own-batch slot select: one contiguous dynamic copy
                    nc.sync.dma_start(lhsrc[h][:], a2a_out[h][ds(rv, 4)])
                    emit_lh_loads(h)

            # ---- tail: h2's proj slices cover the h3 A2A window, then h3's
            # slices stream the output DMAs per (st, nn2) half-row ----
            for j in range(8):
                emit_proj_group(2, j // 2, j % 2)
            for st in range(4):
                for nn2 in range(2):
                    emit_proj_group(3, st, nn2, dma_out=True)

    nc.compile()
    return nc


def _get_compiled():
    global _COMPILED
    if _COMPILED is None:
        _COMPILED = _build()
    return _COMPILED


def make_in_maps(x, attention_mask, w_attn, b_attn, w_proj, b_proj):
    x = np.asarray(x, dtype=np.float32)
    w_attn = np.asarray(w_attn, dtype=np.float32)
    b_attn = np.asarray(b_attn, dtype=np.float32)
    w_proj = np.asarray(w_proj, dtype=np.float32)
    b_proj = np.asarray(b_proj, dtype=np.float32)

    ki, qi = np.meshgrid(np.arange(P), np.arange(P), indexing="ij")
    causalT = np.where(ki > qi, np.float32(0.0), np.float32(1.0))
    # xT [NX, S] -> e-major [NE, P, S], split [:, :, :SC] / [:, :, SC:]
    x8 = [
        np.ascontiguousarray(x[b].T.astype(BF16).reshape(NE, P, S)) for b in range(B)
    ]
    bv_full = b_attn[2 * NX : 3 * NX].astype(np.float64)
    bp_eff = (b_proj.astype(np.float64) + bv_full @ w_proj.astype(np.float64)).astype(
        np.float32
    )
    bp_row32 = np.ascontiguousarray(bp_eff.reshape(1, NX))

    in_maps = []
    for c in range(8):
        b, g = divmod(c, 4)
        cols = slice(HDW * g, HDW * (g + 1))
        kcols = slice(NX + HDW * g, NX + HDW * (g + 1))
        vcols = slice(2 * NX + HDW * g, 2 * NX + HDW * (g + 1))
        bqk_arr = np.concatenate([b_attn[cols] * 0.125, b_attn[kcols]]).reshape(4, P)
        # fi-major q/k weights; the 1/8 score scale is folded into the q
        # columns (exact in bf16: pure exponent shift)
        wqk = np.concatenate(
            [w_attn[:, cols] * 0.125, w_attn[:, kcols]], axis=1
        )  # [NX, 512]
        wqkf = np.ascontiguousarray(
            wqk.reshape(NE, P, 4, P).transpose(2, 1, 0, 3).reshape(4, P, NX)
        ).astype(BF16)
        wvc = np.ascontiguousarray(
            w_attn[:, vcols].reshape(NE, P, HDW).transpose(1, 0, 2).reshape(P, NE * HDW)
        ).astype(BF16)
        # own-batch proj tiles (h, tp): rows 0:64 = in-batch sender 2tp's
        # head-h w_proj rows, 64:128 = sender 2tp+1's
        wtiles = np.zeros((HL, 2, P, NX), dtype=np.float32)
        for h in range(HL):
            for tp in range(2):
                for half, j in ((0, 2 * tp), (1, 2 * tp + 1)):
                    rows = w_proj[HDW * j + D * h : HDW * j + D * (h + 1), :]
                    wtiles[h, tp, 64 * half : 64 * half + D, :] = rows
        wpc = np.ascontiguousarray(
            wtiles.reshape(2 * HL, P, NX).transpose(1, 0, 2).reshape(P, 2 * HL * NX)
        ).astype(BF16)
        in_maps.append(
            {
                "x0": np.ascontiguousarray(x8[b][:, :, :SC]),
                "xr": np.ascontiguousarray(x8[b][:, :, SC:]),
                "wqkf": wqkf,
                "wvc": wvc,
                "wpc": wpc,
                "bqk": np.ascontiguousarray(bqk_arr),
                "bp32": bp_row32,
                "causb": causalT.astype(BF16),
                "onesb": np.ones((P, 4), dtype=BF16),
                "slotb": np.array([[4 * b, 0, 0, 0]], dtype=np.int32),
            }
        )
    return in_maps


def assemble_out(results):
    out = np.empty((B, S, NX), dtype=np.float32)
    for c in range(8):
        b, g = divmod(c, 4)
        out[b, g * SC : (g + 1) * SC, :] = results[c]["out"]
    return out


def run(in_maps, trace=False):
    from concourse.bass_utils import run_bass_kernel_spmd

    nc = _get_compiled()
    return run_bass_kernel_spmd(nc, in_maps, core_ids=list(range(8)), trace=trace)


def kernel(**inputs) -> np.ndarray:
    in_maps = make_in_maps(**inputs)
    res = run(in_maps)
    return assemble_out(res.results)


if __name__ == "__main__":
    _get_compiled()
    print("build+compile OK")


# revision 5
# speedup vs baseline: 1.2477x; 1.2477x over previous
"""Distributed causal multi-head attention for Trainium2 (8 NeuronCores).

Problem: B=2, S=2048, NX=1024, H=16 heads, D=64.
  qkv = x @ w_attn + b_attn ; q,k,v split; causal softmax(q k^T / 8) v ; @ w_proj + b_proj

Sharding: core c -> batch b=c//4 (data parallel), head group g=c%4 (tensor
parallel, 4 heads). Column-split c_attn; after attention four per-head
AllToAlls reshard heads->sequence so each core computes c_proj for its own
512 output rows with the full hidden dim - no cross-core reduction.

Schedule (v2): phase 1 computes only h0/h1's q,k (fi0/fi2) plus most of v;
h2/h3's q,k chains are woven into h0's attention as PE filler. Attention
runs a global 1-block software pipeline: the score matmuls for block
(h,kb+1) are emitted BEFORE the PV of block (h,kb), so ScalarE's exp of
block kb runs concurrently with the next block's score matmuls and the PE
never stalls on exp. All PSUM->SBUF copies during the attention window run
on DVE (tensor_scalar_add / tensor_copy), keeping ScalarE exp-only; the
1/sqrt(D)=1/8 score scale is folded into the q weights host-side (exact in
bf16 - exponent shift). c_proj slices for head h are woven into head h+1's
blocks; all 8 of h2's slices are held back to cover the last A2A's
sync+transfer window, and h3's slices stream per-(st,nn2) output DMAs so
the final HBM writes overlap the remaining matmuls.
"""

import sys

sys.path.insert(0, "/opt/trn_rl_repo")

import numpy as np
import ml_dtypes

BF16 = ml_dtypes.bfloat16

B = 2
S = 2048
NX = 1024
H = 16
D = 64
G = 4            # head groups (tensor-parallel)
HL = H // G      # heads per core = 4
HDW = HL * D     # head-group width = 256
P = 128
SC = 512         # output chunk (A2A granularity)
NQC = S // SC    # 4 chunks
NE = NX // P     # 8 contraction tiles
NKB = S // P     # 16 key blocks
WQ = 1024        # max score-tile width

_COMPILED = None


def _build():
    import concourse.bass as bass  # noqa: F401
    from concourse.bass import ds
    import concourse.mybir as mybir
    import concourse.tile as tile
    from concourse import bacc

    f32 = mybir.dt.float32
    i32 = mybir.dt.int32
    bf16 = mybir.dt.bfloat16
    Identity = mybir.ActivationFunctionType.Identity
    Exp = mybir.ActivationFunctionType.Exp

    nc = bacc.Bacc("TRN2", target_bir_lowering=False, debug=False, num_devices=8)

    x0 = nc.dram_tensor("x0", [NE, P, SC], bf16, kind="ExternalInput")
    xr = nc.dram_tensor("xr", [NE, P, S - SC], bf16, kind="ExternalInput")
    wqkf = nc.dram_tensor("wqkf", [4, P, NX], bf16, kind="ExternalInput")
    wvc = nc.dram_tensor("wvc", [P, NE * HDW], bf16, kind="ExternalInput")
    wpc = nc.dram_tensor("wpc", [P, 2 * HL * NX], bf16, kind="ExternalInput")
    bqk = nc.dram_tensor("bqk", [4, P], f32, kind="ExternalInput")
    bp32 = nc.dram_tensor("bp32", [1, NX], f32, kind="ExternalInput")
    onesb = nc.dram_tensor("onesb", [P, 4], bf16, kind="ExternalInput")
    causb = nc.dram_tensor("causb", [P, P], bf16, kind="ExternalInput")
    slotb = nc.dram_tensor("slotb", [1, 4], i32, kind="ExternalInput")
    out_ext = nc.dram_tensor("out", [SC, NX], f32, kind="ExternalOutput")

    with tile.TileContext(nc) as tc:
        with (
            tc.tile_pool(name="const", bufs=1) as const_pool,
            tc.tile_pool(name="xt", bufs=1) as xt_pool,
            tc.tile_pool(name="w", bufs=1) as w_pool,
            tc.tile_pool(name="qkt", bufs=1) as qkt_pool,
            tc.tile_pool(name="vsb", bufs=1) as v_pool,
            tc.tile_pool(name="lh", bufs=1) as lh_pool,
            tc.tile_pool(name="exp", bufs=12) as exp_pool,
            tc.tile_pool(name="osb", bufs=1) as osb_pool,
            tc.tile_pool(name="small", bufs=3) as small_pool,
            tc.tile_pool(name="wide", bufs=1, space="PSUM") as wide_ps,
            tc.tile_pool(name="scps", bufs=3, space="PSUM") as scps_pool,
            tc.tile_pool(name="atps", bufs=4, space="PSUM") as at_ps_pool,
            tc.tile_pool(name="dram", bufs=1, space="DRAM") as dram_pool,
        ):
            # ---- constants + the batch slot-base register (4*b) ----
            slot_sb = const_pool.tile([1, 4], i32, name="slot_sb")
            nc.sync.dma_start(slot_sb[:], slotb[:])
            sreg = nc.sync.alloc_register("slotreg")
            nc.sync.reg_load(sreg, slot_sb[0:1, 0:1])
            rv = nc.sync.snap(sreg, donate=True, min_val=0, max_val=4)

            bqk_sb = const_pool.tile([P, 4], f32, name="bqk_sb")
            for fi in range(4):
                nc.sync.dma_start(bqk_sb[:, fi : fi + 1], bqk[fi : fi + 1, :])
            bp_sb = const_pool.tile([1, NX], f32, name="bp_sb")
            nc.sync.dma_start(bp_sb[:], bp32[:])
            czb_sb = const_pool.tile([P, P], bf16, name="czb_sb")
            nc.sync.dma_start(czb_sb[:], causb[:])

            # ---- weight + x loads: first-needed first ----
            wqk_sb = {}
            for fi in range(4):
                wqk_sb[fi] = w_pool.tile([P, NX], bf16, name=f"wqk_sb{fi}")
            nc.sync.dma_start(wqk_sb[0][:], wqkf[0])
            nc.sync.dma_start(wqk_sb[2][:], wqkf[2])
            xt0_sb = {}
            wv_sb = {}
            for e in range(NE):
                t = xt_pool.tile([P, SC], bf16, name=f"xt0_{e}")
                nc.sync.dma_start(t[:], x0[e])
                xt0_sb[e] = t
                tv = w_pool.tile([P, HDW], bf16, name=f"wv_sb{e}")
                nc.sync.dma_start(tv[:], wvc[:, e * HDW : (e + 1) * HDW])
                wv_sb[e] = tv
            xtr_sb = {}
            for e in range(NE):
                t = xt_pool.tile([P, S - SC], bf16, name=f"xtr_{e}")
                nc.sync.dma_start(t[:], xr[e])
                xtr_sb[e] = t
            nc.sync.dma_start(wqk_sb[1][:], wqkf[1])
            nc.sync.dma_start(wqk_sb[3][:], wqkf[3])
            wp_sb = w_pool.tile([P, 2 * HL * NX], bf16, name="wp_sb")
            nc.sync.dma_start(wp_sb[:], wpc[:])

            def xt_slice(e, c0, w):
                if c0 < SC:
                    return xt0_sb[e][:, c0 : c0 + w]
                return xtr_sb[e][:, c0 - SC : c0 - SC + w]

            # ---- persistent q/k/v SBUF state ----
            qkt_sb = {}
            for fi in range(2):
                qkt_sb[fi] = qkt_pool.tile(
                    [P, S], bf16, name=f"qkt{fi}", tag=f"qktw{fi}"
                )
            # per-head kT with the other head's rows zeroed: score matmuls
            # run at K=128 (zeros annihilate the foreign q rows), keeping
            # switching activity low for the HAM power governor
            ktz_sb = {}
            for h in range(HL):
                ktz_sb[h] = qkt_pool.tile([P, S], bf16, name=f"ktz{h}", tag=f"ktz{h}")
                nc.gpsimd.memset(ktz_sb[h][:], 0.0)
            v_sb = {}

            # ---- chain emitters (qk / v); copies on ACT in phase 1,
            # DVE during the attention window so ScalarE stays exp-only ----
            def emit_qk_chain(fi, sc, on_act):
                ps = at_ps_pool.tile(
                    [P, SC], f32, tag="atps", name=f"qk_ps{fi}_{sc}"
                )
                for e in range(NE):
                    nc.tensor.matmul(
                        ps[:],
                        wqk_sb[fi][:, e * P : (e + 1) * P],
                        xt_slice(e, sc * SC, SC),
                        start=(e == 0),
                        stop=(e == NE - 1),
                    )
                cols = slice(sc * SC, (sc + 1) * SC)
                if fi < 2:
                    if on_act:
                        nc.scalar.activation(
                            qkt_sb[fi][:, cols], ps[:], Identity,
                            bias=bqk_sb[:, fi : fi + 1],
                        )
                    else:
                        nc.vector.tensor_scalar_add(
                            qkt_sb[fi][:, cols], ps[:], bqk_sb[:, fi : fi + 1]
                        )
                else:
                    for hh in range(2):
                        h = 2 * (fi - 2) + hh
                        r0 = 64 * hh
                        if on_act:
                            nc.scalar.activation(
                                ktz_sb[h][r0 : r0 + D, cols],
                                ps[r0 : r0 + D, :],
                                Identity,
                                bias=bqk_sb[r0 : r0 + D, fi : fi + 1],
                            )
                        else:
                            nc.vector.tensor_scalar_add(
                                ktz_sb[h][r0 : r0 + D, cols],
                                ps[r0 : r0 + D, :],
                                bqk_sb[r0 : r0 + D, fi : fi + 1],
                            )

            def emit_v_chain(si, on_act):
                sc, j = divmod(si, 4)
                psv = wide_ps.tile([P, HDW], f32, tag="wide", name=f"v_ps{si}")
                for e in range(NE):
                    nc.tensor.matmul(
                        psv[:],
                        xt_slice(e, sc * SC + j * P, P),
                        wv_sb[e][:],
                        start=(e == 0),
                        stop=(e == NE - 1),
                    )
                # per-head 128-wide slots: [v(64) | ones(1) | zeros(63)]
                vt = v_pool.tile([P, HL * P], bf16, name=f"v{si}")
                nc.gpsimd.memset(vt[:], 0.0)
                nc.sync.dma_start(
                    vt[:].rearrange("p (h u) -> p h u", h=HL)[:, :, D : D + 1],
                    onesb[:],
                )
                dst = vt[:].rearrange("p (h u) -> p h u", h=HL)[:, :, 0:D]
                src = psv[:].rearrange("p (h u) -> p h u", h=HL)
                if on_act:
                    nc.scalar.activation(dst, src, Identity)
                else:
                    nc.vector.tensor_copy(dst, src)
                v_sb[si] = vt

            # ---- phase 1a: fi0/fi2 q,k + v for token blocks 0..11;
            # sc0 first (x0-only work) so the xtr DMAs can land ----
            emit_qk_chain(0, 0, True)
            emit_v_chain(0, True)
            emit_qk_chain(2, 0, True)
            emit_v_chain(1, True)
            emit_v_chain(2, True)
            emit_v_chain(3, True)
            emit_qk_chain(0, 1, True)
            emit_v_chain(4, True)
            emit_qk_chain(2, 1, True)
            emit_v_chain(5, True)
            emit_v_chain(6, True)
            emit_v_chain(7, True)
            emit_qk_chain(0, 2, True)
            emit_v_chain(8, True)
            emit_qk_chain(2, 2, True)
            emit_v_chain(9, True)
            emit_v_chain(10, True)
            emit_v_chain(11, True)
            emit_qk_chain(0, 3, True)
            emit_qk_chain(2, 3, True)

            # ---- A2A buffers ----
            a2a_in = {}
            a2a_out = {}
            lhsrc = {}
            for h in range(HL):
                a2a_in[h] = dram_pool.tile([8, D, SC], bf16, name=f"a2a_in{h}")
                a2a_out[h] = dram_pool.tile([8, D, SC], bf16, name=f"a2a_out{h}")
                lhsrc[h] = dram_pool.tile([4, D, SC], bf16, name=f"lhsrc{h}")

            def emit_tail(h, qc, at_ps):
                # softmax denominator comes from the ones-column of v; the
                # v-bias is folded into bp on the host
                dn32 = small_pool.tile([1, SC], f32, tag="dn32", name=f"dn32{qc}_{h}")
                nc.vector.tensor_copy(dn32[:], at_ps[D : D + 1, :])
                rc32 = small_pool.tile([1, SC], f32, tag="rc32", name=f"rc32{qc}_{h}")
                nc.vector.reciprocal_approx_fast(rc32[:], dn32[:])
                rb = small_pool.tile([D, SC], f32, tag="rbsb", name=f"rbsb{qc}_{h}")
                nc.gpsimd.partition_broadcast(rb[:], rc32[:])
                ath = small_pool.tile(
                    [D, SC], bf16, tag="ath", bufs=3, name=f"ath{qc}_{h}"
                )
                nc.vector.tensor_mul(ath[:], at_ps[0:D, :], rb[:])
                nc.sync.dma_start(a2a_in[h][qc, :, :], ath[:])
                nc.sync.dma_start(a2a_in[h][qc + 4, :, :], ath[:])

            def emit_score_exp(h, kb):
                # score tiles (transposed [k, q]) + exp for key block kb;
                # 512-wide pieces through the dedicated 3-deep scps pool so
                # the next block's score matmuls never serialize on this
                # block's exp
                fi_q = h // 2
                q0 = P * kb
                out = []
                for s2 in range((S - q0 + SC - 1) // SC):
                    w0 = q0 + s2 * SC
                    ww = min(SC, S - w0)
                    scp = scps_pool.tile(
                        [P, SC], f32, tag="scps", name=f"sc{h}_{kb}_{s2}"
                    )
                    nc.tensor.matmul(
                        scp[:, 0:ww],
                        ktz_sb[h][:, q0 : q0 + P],
                        qkt_sb[fi_q][:, w0 : w0 + ww],
                        start=True,
                        stop=True,
                    )
                    ex = exp_pool.tile(
                        [P, SC], bf16, tag="exp", name=f"ex{h}_{kb}_{s2}"
                    )
                    nc.scalar.activation(ex[:, 0:ww], scp[:, 0:ww], Exp)
                    if s2 == 0:
                        # diagonal block: zero the non-causal upper triangle
                        nc.vector.tensor_mul(ex[:, 0:P], ex[:, 0:P], czb_sb[:])
                    out.append((ex, w0, ww))
                return out

            def emit_pv(h, kb, exl, at_ps):
                for ex, w0, ww in exl:
                    qc_lo = w0 // SC
                    qc_hi = (w0 + ww - 1) // SC
                    for qc in range(qc_lo, qc_hi + 1):
                        a0 = max(w0, qc * SC)
                        a1 = min(w0 + ww, (qc + 1) * SC)
                        nc.tensor.matmul(
                            at_ps[qc][:, a0 - qc * SC : a1 - qc * SC],
                            v_sb[kb][:, h * P : (h + 1) * P],
                            ex[:, a0 - w0 : a1 - w0],
                            start=(kb == 0),
                            stop=(kb == 4 * qc + 3),
                        )

            # ---- c_proj state + emitters ----
            osb = {}
            for st in range(4):
                osb[st] = osb_pool.tile([P, NX], f32, name=f"osb{st}")
                nc.gpsimd.partition_broadcast(osb[st][:], bp_sb[:])
            lh = {}

            def emit_lh_loads(h):
                for tp in range(2):
                    t = lh_pool.tile([P, SC], bf16, name=f"lh{h}_{tp}")
                    # two half-tile loads land on different DMA queues
                    nc.sync.dma_start(t[0:D, :], lhsrc[h][2 * tp])
                    nc.sync.dma_start(t[D:P, :], lhsrc[h][2 * tp + 1])
                    lh[h, tp] = t

            def emit_proj_group(h, st, nn2, dma_out=False):
                pp = wide_ps.tile([P, SC], f32, tag="wide", name=f"pj{h}_{st}_{nn2}")
                for tp in range(2):
                    wcol = (2 * h + tp) * NX + nn2 * SC
                    nc.tensor.matmul(
                        pp[:],
                        lh[h, tp][:, st * P : (st + 1) * P],
                        wp_sb[:, wcol : wcol + SC],
                        start=(tp == 0),
                        stop=(tp == 1),
                    )
                dst = osb[st][:, nn2 * SC : (nn2 + 1) * SC]
                nc.vector.tensor_add(dst, dst, pp[:])
                if dma_out:
                    nc.sync.dma_start(
                        out_ext[st * P : (st + 1) * P, nn2 * SC : (nn2 + 1) * SC],
                        dst,
                    )

            # ---- dummy A2A: resyncs core drift before the real A2As and
            # absorbs the collective stream's one-time setup cost ----
            a2ad_in = dram_pool.tile([8, 1, P], bf16, name="a2ad_in")
            a2ad_out = dram_pool.tile([8, 1, P], bf16, name="a2ad_out")
            for s in range(8):
                nc.sync.dma_start(a2ad_in[s], v_sb[11][0:1, 0:P])
            nc.gpsimd.collective_compute(
                "AllToAll",
                mybir.AluOpType.bypass,
                ins=[a2ad_in[:].opt()],
                outs=[a2ad_out[:].opt()],
                replica_groups=[list(range(8))],
            )

            # ---- fill schedule: work woven between score(kb+1) and pv(kb).
            # h0: remaining v chains + h2/h3's q,k chains (copies on DVE);
            # h1: h0's proj slices; h2: h1's; h3: lean (h2's slices are the
            # reserve that covers the final A2A window).
            fills = {}
            for j in range(4):
                fills[0, j] = ("v", 12 + j)
            for sc in range(4):
                fills[0, 4 + sc] = ("qk", 1, sc)
            for sc in range(4):
                fills[0, 8 + sc] = ("qk", 3, sc)
            for j in range(8):
                fills[1, 4 + j] = ("proj", 0, j // 2, j % 2)
            for j in range(8):
                fills[2, 4 + j] = ("proj", 1, j // 2, j % 2)

            def emit_fill(f):
                if f[0] == "v":
                    emit_v_chain(f[1], on_act=False)
                elif f[0] == "qk":
                    emit_qk_chain(f[1], f[2], on_act=False)
                else:
                    emit_proj_group(f[1], f[2], f[3])

            # ---- attention: global 1-block software pipeline ----
            blocks = [(h, kb) for h in range(HL) for kb in range(NKB)]
            at_ps_all = {}
            for h in range(HL):
                at_ps_all[h] = {}
            exl_next = emit_score_exp(0, 0)
            for i, (h, kb) in enumerate(blocks):
                if kb == 0:
                    for qc in range(NQC):
                        at_ps_all[h][qc] = at_ps_pool.tile(
                            [P, SC], f32, tag="atps", name=f"at_ps{qc}_{h}"
                        )
                exl = exl_next
                if i + 1 < len(blocks):
                    exl_next = emit_score_exp(*blocks[i + 1])
                f = fills.get((h, kb))
                if f is not None:
                    emit_fill(f)
                emit_pv(h, kb, exl, at_ps_all[h])
                for qc in range(NQC):
                    if kb == 4 * qc + 3:
                        emit_tail(h, qc, at_ps_all[h][qc])
                if kb == NKB - 1:
                    nc.gpsimd.collective_compute(
                        "AllToAll",
                        mybir.AluOpType.bypass,
                        ins=[a2a_in[h][:].opt()],
                        outs=[a2a_out[h][:].opt()],
                        replica_groups=[list(range(8))],
                    )
                    # own-batch slot select: one contiguous dynamic copy
                    nc.sync.dma_start(lhsrc[h][:], a2a_out[h][ds(rv, 4)])
                    emit_lh_loads(h)

            # ---- tail: h2's proj slices cover the h3 A2A window, then h3's
            # slices stream the output DMAs per (st, nn2) half-row ----
            for j in range(8):
                emit_proj_group(2, j // 2, j % 2)
            for st in range(4):
                for nn2 in range(2):
                    emit_proj_group(3, st, nn2, dma_out=True)

    nc.compile()
    return nc


def _get_compiled():
    global _COMPILED
    if _COMPILED is None:
        _COMPILED = _build()
    return _COMPILED


def make_in_maps(x, attention_mask, w_attn, b_attn, w_proj, b_proj):
    x = np.asarray(x, dtype=np.float32)
    w_attn = np.asarray(w_attn, dtype=np.float32)
    b_attn = np.asarray(b_attn, dtype=np.float32)
    w_proj = np.asarray(w_proj, dtype=np.float32)
    b_proj = np.asarray(b_proj, dtype=np.float32)

    ki, qi = np.meshgrid(np.arange(P), np.arange(P), indexing="ij")
    causalT = np.where(ki > qi, np.float32(0.0), np.float32(1.0))
    # xT [NX, S] -> e-major [NE, P, S], split [:, :, :SC] / [:, :, SC:]
    x8 = [
        np.ascontiguousarray(x[b].T.astype(BF16).reshape(NE, P, S)) for b in range(B)
    ]
    bv_full = b_attn[2 * NX : 3 * NX].astype(np.float64)
    bp_eff = (b_proj.astype(np.float64) + bv_full @ w_proj.astype(np.float64)).astype(
        np.float32
    )
    bp_row32 = np.ascontiguousarray(bp_eff.reshape(1, NX))

    in_maps = []
    for c in range(8):
        b, g = divmod(c, 4)
        cols = slice(HDW * g, HDW * (g + 1))
        kcols = slice(NX + HDW * g, NX + HDW * (g + 1))
        vcols = slice(2 * NX + HDW * g, 2 * NX + HDW * (g + 1))
        bqk_arr = np.concatenate([b_attn[cols] * 0.125, b_attn[kcols]]).reshape(4, P)
        # fi-major q/k weights; the 1/8 score scale is folded into the q
        # columns (exact in bf16: pure exponent shift)
        wqk = np.concatenate(
            [w_attn[:, cols] * 0.125, w_attn[:, kcols]], axis=1
        )  # [NX, 512]
        wqkf = np.ascontiguousarray(
            wqk.reshape(NE, P, 4, P).transpose(2, 1, 0, 3).reshape(4, P, NX)
        ).astype(BF16)
        wvc = np.ascontiguousarray(
            w_attn[:, vcols].reshape(NE, P, HDW).transpose(1, 0, 2).reshape(P, NE * HDW)
        ).astype(BF16)
        # own-batch proj tiles (h, tp): rows 0:64 = in-batch sender 2tp's
        # head-h w_proj rows, 64:128 = sender 2tp+1's
        wtiles = np.zeros((HL, 2, P, NX), dtype=np.float32)
        for h in range(HL):
            for tp in range(2):
                for half, j in ((0, 2 * tp), (1, 2 * tp + 1)):
                    rows = w_proj[HDW * j + D * h : HDW * j + D * (h + 1), :]
                    wtiles[h, tp, 64 * half : 64 * half + D, :] = rows
        wpc = np.ascontiguousarray(
            wtiles.reshape(2 * HL, P, NX).transpose(1, 0, 2).reshape(P, 2 * HL * NX)
        ).astype(BF16)
        in_maps.append(
            {
                "x0": np.ascontiguousarray(x8[b][:, :, :SC]),
                "xr": np.ascontiguousarray(x8[b][:, :, SC:]),
                "wqkf": wqkf,
                "wvc": wvc,
                "wpc": wpc,
                "bqk": np.ascontiguousarray(bqk_arr),
                "bp32": bp_row32,
                "causb": causalT.astype(BF16),
                "onesb": np.ones((P, 4), dtype=BF16),
                "slotb": np.array([[4 * b, 0, 0, 0]], dtype=np.int32),
            }
        )
    return in_maps


def assemble_out(results):
    out = np.empty((B, S, NX), dtype=np.float32)
    for c in range(8):
        b, g = divmod(c, 4)
        out[b, g * SC : (g + 1) * SC, :] = results[c]["out"]
    return out


def run(in_maps, trace=False):
    from concourse.bass_utils import run_bass_kernel_spmd

    nc = _get_compiled()
    return run_bass_kernel_spmd(nc, in_maps, core_ids=list(range(8)), trace=trace)


def kernel(**inputs) -> np.ndarray:
    in_maps = make_in_maps(**inputs)
    res = run(in_maps)
    return assemble_out(res.results)


if __name__ == "__main__":
    _get_compiled()
    print("build+compile OK")


# revision 12
# speedup vs baseline: 1.2976x; 1.0400x over previous
"""Distributed causal multi-head attention for Trainium2 (8 NeuronCores).

Problem: B=2, S=2048, NX=1024, H=16 heads, D=64.
  qkv = x @ w_attn + b_attn ; q,k,v split; causal softmax(q k^T / 8) v ; @ w_proj + b_proj

Sharding: core c -> batch b=c//4 (data parallel), head group g=c%4 (tensor
parallel, 4 heads). Column-split c_attn; after attention four per-head
AllToAlls reshard heads->sequence so each core computes c_proj for its own
512 output rows with the full hidden dim - no cross-core reduction.

Schedule (v2): phase 1 computes only h0/h1's q,k (fi0/fi2) plus most of v;
h2/h3's q,k chains are woven into h0's attention as PE filler. Attention
runs a global 1-block software pipeline: the score matmuls for block
(h,kb+1) are emitted BEFORE the PV of block (h,kb), so ScalarE's exp of
block kb runs concurrently with the next block's score matmuls and the PE
never stalls on exp. All PSUM->SBUF copies during the attention window run
on DVE (tensor_scalar_add / tensor_copy), keeping ScalarE exp-only; the
1/sqrt(D)=1/8 score scale is folded into the q weights host-side (exact in
bf16 - exponent shift). c_proj slices for head h are woven into head h+1's
blocks; all 8 of h2's slices are held back to cover the last A2A's
sync+transfer window, and h3's slices stream per-(st,nn2) output DMAs so
the final HBM writes overlap the remaining matmuls.
"""

import sys

sys.path.insert(0, "/opt/trn_rl_repo")

import numpy as np
import ml_dtypes

BF16 = ml_dtypes.bfloat16

B = 2
S = 2048
NX = 1024
H = 16
D = 64
G = 4            # head groups (tensor-parallel)
HL = H // G      # heads per core = 4
HDW = HL * D     # head-group width = 256
P = 128
SC = 512         # output chunk (A2A granularity)
NQC = S // SC    # 4 chunks
NE = NX // P     # 8 contraction tiles
NKB = S // P     # 16 key blocks
WQ = 1024        # max score-tile width

_COMPILED = None


def _build():
    import concourse.bass as bass  # noqa: F401
    from concourse.bass import ds
    import concourse.mybir as mybir
    import concourse.tile as tile
    from concourse import bacc

    f32 = mybir.dt.float32
    i32 = mybir.dt.int32
    bf16 = mybir.dt.bfloat16
    Identity = mybir.ActivationFunctionType.Identity
    Exp = mybir.ActivationFunctionType.Exp

    nc = bacc.Bacc("TRN2", target_bir_lowering=False, debug=False, num_devices=8)

    x0 = nc.dram_tensor("x0", [NE, P, SC], bf16, kind="ExternalInput")
    xr = nc.dram_tensor("xr", [NE, P, S - SC], bf16, kind="ExternalInput")
    wqkf = nc.dram_tensor("wqkf", [4, P, NX], bf16, kind="ExternalInput")
    wvc = nc.dram_tensor("wvc", [P, NE * HDW], bf16, kind="ExternalInput")
    wpc = nc.dram_tensor("wpc", [P, 2 * HL * NX], bf16, kind="ExternalInput")
    bqk = nc.dram_tensor("bqk", [4, P], f32, kind="ExternalInput")
    bp32 = nc.dram_tensor("bp32", [1, NX], f32, kind="ExternalInput")
    onesb = nc.dram_tensor("onesb", [P, 4], bf16, kind="ExternalInput")
    causb = nc.dram_tensor("causb", [P, P], bf16, kind="ExternalInput")
    slotb = nc.dram_tensor("slotb", [1, 4], i32, kind="ExternalInput")
    out_ext = nc.dram_tensor("out", [SC, NX], f32, kind="ExternalOutput")

    with tile.TileContext(nc) as tc:
        with (
            tc.tile_pool(name="const", bufs=1) as const_pool,
            tc.tile_pool(name="xt", bufs=1) as xt_pool,
            tc.tile_pool(name="w", bufs=1) as w_pool,
            tc.tile_pool(name="qkt", bufs=1) as qkt_pool,
            tc.tile_pool(name="vsb", bufs=1) as v_pool,
            tc.tile_pool(name="lh", bufs=1) as lh_pool,
            tc.tile_pool(name="exp", bufs=12) as exp_pool,
            tc.tile_pool(name="osb", bufs=1) as osb_pool,
            tc.tile_pool(name="small", bufs=3) as small_pool,
            tc.tile_pool(name="wide", bufs=1, space="PSUM") as wide_ps,
            tc.tile_pool(name="scps", bufs=3, space="PSUM") as scps_pool,
            tc.tile_pool(name="atps", bufs=4, space="PSUM") as at_ps_pool,
            tc.tile_pool(name="dram", bufs=1, space="DRAM") as dram_pool,
        ):
            # ---- constants + per-batch predicate registers (1/0) ----
            slot_sb = const_pool.tile([1, 4], i32, name="slot_sb")
            nc.sync.dma_start(slot_sb[:], slotb[:])
            r0reg = nc.sync.alloc_register("isb0reg")
            nc.sync.reg_load(r0reg, slot_sb[0:1, 0:1])
            rb0 = nc.sync.snap(r0reg, donate=True, min_val=0, max_val=1)
            r1reg = nc.sync.alloc_register("isb1reg")
            nc.sync.reg_load(r1reg, slot_sb[0:1, 1:2])
            rb1 = nc.sync.snap(r1reg, donate=True, min_val=0, max_val=1)

            bqk_sb = const_pool.tile([P, 4], f32, name="bqk_sb")
            for fi in range(4):
                nc.sync.dma_start(bqk_sb[:, fi : fi + 1], bqk[fi : fi + 1, :])
            bp_sb = const_pool.tile([1, NX], f32, name="bp_sb")
            nc.sync.dma_start(bp_sb[:], bp32[:])
            czb_sb = const_pool.tile([P, P], bf16, name="czb_sb")
            nc.sync.dma_start(czb_sb[:], causb[:])

            # ---- weight + x loads: first-needed first ----
            wqk_sb = {}
            for fi in range(4):
                wqk_sb[fi] = w_pool.tile([P, NX], bf16, name=f"wqk_sb{fi}")
            # wqk0 in per-e pieces: the first matmul's weight dep is 32KB on
            # its own DMA queue instead of a 256KB monolith sharing bandwidth
            for e in range(NE):
                nc.sync.dma_start(
                    wqk_sb[0][:, e * P : (e + 1) * P], wqkf[0][:, e * P : (e + 1) * P]
                )
            xt0_sb = {}
            wv_sb = {}
            for e in range(NE):
                t = xt_pool.tile([P, SC], bf16, name=f"xt0_{e}")
                nc.sync.dma_start(t[:], x0[e])
                xt0_sb[e] = t
                tv = w_pool.tile([P, HDW], bf16, name=f"wv_sb{e}")
                nc.sync.dma_start(tv[:], wvc[:, e * HDW : (e + 1) * HDW])
                wv_sb[e] = tv
                if e == 3:
                    nc.sync.dma_start(wqk_sb[2][:], wqkf[2])
            xtr_sb = {}
            for e in range(NE):
                t = xt_pool.tile([P, S - SC], bf16, name=f"xtr_{e}")
                nc.sync.dma_start(t[:], xr[e])
                xtr_sb[e] = t
            nc.sync.dma_start(wqk_sb[1][:], wqkf[1])
            nc.sync.dma_start(wqk_sb[3][:], wqkf[3])
            wp_sb = w_pool.tile([P, 2 * HL * NX], bf16, name="wp_sb")
            nc.sync.dma_start(wp_sb[:], wpc[:])

            def xt_slice(e, c0, w):
                if c0 < SC:
                    return xt0_sb[e][:, c0 : c0 + w]
                return xtr_sb[e][:, c0 - SC : c0 - SC + w]

            # ---- persistent q/k/v SBUF state ----
            qkt_sb = {}
            for fi in range(2):
                qkt_sb[fi] = qkt_pool.tile(
                    [P, S], bf16, name=f"qkt{fi}", tag=f"qktw{fi}"
                )
            # per-head kT with the other head's rows zeroed: score matmuls
            # run at K=128 (zeros annihilate the foreign q rows), keeping
            # switching activity low for the HAM power governor
            ktz_sb = {}
            for h in range(HL):
                ktz_sb[h] = qkt_pool.tile([P, S], bf16, name=f"ktz{h}", tag=f"ktz{h}")
                nc.gpsimd.memset(ktz_sb[h][:], 0.0)
            v_sb = {}

            # ---- chain emitters (qk / v); copies on ACT in phase 1,
            # DVE during the attention window so ScalarE stays exp-only ----
            def emit_qk_chain(fi, sc, on_act):
                ps = at_ps_pool.tile(
                    [P, SC], f32, tag="atps", name=f"qk_ps{fi}_{sc}"
                )
                for e in range(NE):
                    nc.tensor.matmul(
                        ps[:],
                        wqk_sb[fi][:, e * P : (e + 1) * P],
                        xt_slice(e, sc * SC, SC),
                        start=(e == 0),
                        stop=(e == NE - 1),
                    )
                cols = slice(sc * SC, (sc + 1) * SC)
                if fi < 2:
                    if on_act:
                        nc.scalar.activation(
                            qkt_sb[fi][:, cols], ps[:], Identity,
                            bias=bqk_sb[:, fi : fi + 1],
                        )
                    else:
                        nc.vector.tensor_scalar_add(
                            qkt_sb[fi][:, cols], ps[:], bqk_sb[:, fi : fi + 1]
                        )
                else:
                    for hh in range(2):
                        h = 2 * (fi - 2) + hh
                        r0 = 64 * hh
                        if on_act:
                            nc.scalar.activation(
                                ktz_sb[h][r0 : r0 + D, cols],
                                ps[r0 : r0 + D, :],
                                Identity,
                                bias=bqk_sb[r0 : r0 + D, fi : fi + 1],
                            )
                        else:
                            nc.vector.tensor_scalar_add(
                                ktz_sb[h][r0 : r0 + D, cols],
                                ps[r0 : r0 + D, :],
                                bqk_sb[r0 : r0 + D, fi : fi + 1],
                            )

            def emit_v_chain(si, on_act):
                sc, j = divmod(si, 4)
                psv = wide_ps.tile([P, HDW], f32, tag="wide", name=f"v_ps{si}")
                for e in range(NE):
                    nc.tensor.matmul(
                        psv[:],
                        xt_slice(e, sc * SC + j * P, P),
                        wv_sb[e][:],
                        start=(e == 0),
                        stop=(e == NE - 1),
                    )
                # per-head 128-wide slots: [v(64) | ones(1) | zeros(63)]
                vt = v_pool.tile([P, HL * P], bf16, name=f"v{si}")
                nc.gpsimd.memset(vt[:], 0.0)
                nc.sync.dma_start(
                    vt[:].rearrange("p (h u) -> p h u", h=HL)[:, :, D : D + 1],
                    onesb[:],
                )
                dst = vt[:].rearrange("p (h u) -> p h u", h=HL)[:, :, 0:D]
                src = psv[:].rearrange("p (h u) -> p h u", h=HL)
                if on_act:
                    nc.scalar.activation(dst, src, Identity)
                else:
                    nc.vector.tensor_copy(dst, src)
                v_sb[si] = vt

            # ---- phase 1a: fi0/fi2 q,k + v for token blocks 0..11;
            # sc0 first (x0-only work) so the xtr DMAs can land ----
            emit_qk_chain(0, 0, True)
            emit_v_chain(0, True)
            emit_qk_chain(2, 0, True)
            emit_v_chain(1, True)
            emit_v_chain(2, True)
            emit_v_chain(3, True)
            emit_qk_chain(0, 1, True)
            emit_v_chain(4, True)
            emit_qk_chain(2, 1, True)
            emit_v_chain(5, True)
            emit_v_chain(6, True)
            emit_v_chain(7, True)
            emit_qk_chain(0, 2, True)
            emit_v_chain(8, True)
            emit_qk_chain(2, 2, True)
            emit_v_chain(9, True)
            emit_v_chain(10, True)
            emit_v_chain(11, True)
            emit_qk_chain(0, 3, True)
            emit_qk_chain(2, 3, True)

            # ---- A2A buffers ----
            a2a_in = {}
            a2a_out = {}
            for h in range(HL):
                a2a_in[h] = dram_pool.tile([8, D, SC], bf16, name=f"a2a_in{h}")
                a2a_out[h] = dram_pool.tile([8, D, SC], bf16, name=f"a2a_out{h}")

            def emit_tail(h, qc, at_ps):
                # softmax denominator comes from the ones-column of v; the
                # v-bias is folded into bp on the host
                dn32 = small_pool.tile([1, SC], f32, tag="dn32", name=f"dn32{qc}_{h}")
                nc.vector.tensor_copy(dn32[:], at_ps[D : D + 1, :])
                rc32 = small_pool.tile([1, SC], f32, tag="rc32", name=f"rc32{qc}_{h}")
                nc.vector.reciprocal_approx_fast(rc32[:], dn32[:])
                rb = small_pool.tile([D, SC], f32, tag="rbsb", name=f"rbsb{qc}_{h}")
                nc.gpsimd.partition_broadcast(rb[:], rc32[:])
                ath = small_pool.tile(
                    [D, SC], bf16, tag="ath", bufs=3, name=f"ath{qc}_{h}"
                )
                nc.vector.tensor_mul(ath[:], at_ps[0:D, :], rb[:])
                nc.sync.dma_start(a2a_in[h][qc, :, :], ath[:])
                nc.sync.dma_start(a2a_in[h][qc + 4, :, :], ath[:])

            def emit_score_exp(h, kb):
                # score tiles (transposed [k, q]) + exp for key block kb;
                # 512-wide pieces through the dedicated 3-deep scps pool so
                # the next block's score matmuls never serialize on this
                # block's exp
                fi_q = h // 2
                q0 = P * kb
                out = []
                for s2 in range((S - q0 + SC - 1) // SC):
                    w0 = q0 + s2 * SC
                    ww = min(SC, S - w0)
                    scp = scps_pool.tile(
                        [P, SC], f32, tag="scps", name=f"sc{h}_{kb}_{s2}"
                    )
                    nc.tensor.matmul(
                        scp[:, 0:ww],
                        ktz_sb[h][:, q0 : q0 + P],
                        qkt_sb[fi_q][:, w0 : w0 + ww],
                        start=True,
                        stop=True,
                    )
                    ex = exp_pool.tile(
                        [P, SC], bf16, tag="exp", name=f"ex{h}_{kb}_{s2}"
                    )
                    nc.scalar.activation(ex[:, 0:ww], scp[:, 0:ww], Exp)
                    if s2 == 0:
                        # diagonal block: zero the non-causal upper triangle
                        nc.vector.tensor_mul(ex[:, 0:P], ex[:, 0:P], czb_sb[:])
                    out.append((ex, w0, ww))
                return out

            def emit_pv(h, kb, exl, at_ps):
                for ex, w0, ww in exl:
                    qc_lo = w0 // SC
                    qc_hi = (w0 + ww - 1) // SC
                    for qc in range(qc_lo, qc_hi + 1):
                        a0 = max(w0, qc * SC)
                        a1 = min(w0 + ww, (qc + 1) * SC)
                        nc.tensor.matmul(
                            at_ps[qc][:, a0 - qc * SC : a1 - qc * SC],
                            v_sb[kb][:, h * P : (h + 1) * P],
                            ex[:, a0 - w0 : a1 - w0],
                            start=(kb == 0),
                            stop=(kb == 4 * qc + 3),
                        )

            # ---- c_proj state + emitters ----
            osb = {}
            for st in range(4):
                osb[st] = osb_pool.tile([P, NX], f32, name=f"osb{st}")
                nc.gpsimd.partition_broadcast(osb[st][:], bp_sb[:])
            lh = {}

            def emit_lh_loads(h):
                # predicated static loads straight from a2a_out: the own-batch
                # slot pair (4b+2tp, 4b+2tp+1) is 128KB contiguous; emit both
                # batch variants, exactly one runs (cond register), killing the
                # dynamic DRAM->DRAM slot-select copy and its latency
                for tp in range(2):
                    t = lh_pool.tile([P, SC], bf16, name=f"lh{h}_{tp}")
                    for bvar, rb in ((0, rb0), (1, rb1)):
                        src = a2a_out[h][4 * bvar + 2 * tp : 4 * bvar + 2 * tp + 2]
                        nc.sync.dma_start(
                            t[:], src.rearrange("a d c -> (a d) c"), cond=rb
                        )
                    lh[h, tp] = t

            def emit_proj_group(h, st, nn2, dma_out=False):
                pp = wide_ps.tile([P, SC], f32, tag="wide", name=f"pj{h}_{st}_{nn2}")
                for tp in range(2):
                    wcol = (2 * h + tp) * NX + nn2 * SC
                    nc.tensor.matmul(
                        pp[:],
                        lh[h, tp][:, st * P : (st + 1) * P],
                        wp_sb[:, wcol : wcol + SC],
                        start=(tp == 0),
                        stop=(tp == 1),
                    )
                dst = osb[st][:, nn2 * SC : (nn2 + 1) * SC]
                nc.vector.tensor_add(dst, dst, pp[:])
                if dma_out:
                    nc.sync.dma_start(
                        out_ext[st * P : (st + 1) * P, nn2 * SC : (nn2 + 1) * SC],
                        dst,
                    )

            # ---- dummy A2A: resyncs core drift before the real A2As and
            # absorbs the collective stream's one-time setup cost ----
            a2ad_in = dram_pool.tile([8, 1, P], bf16, name="a2ad_in")
            a2ad_out = dram_pool.tile([8, 1, P], bf16, name="a2ad_out")
            for s in range(8):
                nc.sync.dma_start(a2ad_in[s], v_sb[11][0:1, 0:P])
            nc.gpsimd.collective_compute(
                "AllToAll",
                mybir.AluOpType.bypass,
                ins=[a2ad_in[:].opt()],
                outs=[a2ad_out[:].opt()],
                replica_groups=[list(range(8))],
            )

            # ---- fill schedule: work woven between score(kb+1) and pv(kb).
            # h0: remaining v chains + h2/h3's q,k chains (copies on DVE);
            # h1: h0's proj slices; h2: h1's; h3: lean (h2's slices are the
            # reserve that covers the final A2A window).
            fills = {}
            for j in range(4):
                fills[0, j] = ("v", 12 + j)
            for sc in range(4):
                fills[0, 4 + sc] = ("qk", 1, sc)
            for sc in range(4):
                fills[0, 8 + sc] = ("qk", 3, sc)
            for j in range(8):
                fills[1, 4 + j] = ("proj", 0, j // 2, j % 2)

            def emit_fill(f):
                if f[0] == "v":
                    emit_v_chain(f[1], on_act=False)
                elif f[0] == "qk":
                    emit_qk_chain(f[1], f[2], on_act=False)
                else:
                    emit_proj_group(f[1], f[2], f[3])

            # ---- attention: global 1-block software pipeline ----
            blocks = [(h, kb) for h in range(HL) for kb in range(NKB)]
            at_ps_all = {}
            for h in range(HL):
                at_ps_all[h] = {}
            exl_next = emit_score_exp(0, 0)
            for i, (h, kb) in enumerate(blocks):
                if kb == 0:
                    for qc in range(NQC):
                        at_ps_all[h][qc] = at_ps_pool.tile(
                            [P, SC], f32, tag="atps", name=f"at_ps{qc}_{h}"
                        )
                exl = exl_next
                if i + 1 < len(blocks):
                    exl_next = emit_score_exp(*blocks[i + 1])
                f = fills.get((h, kb))
                if f is not None:
                    emit_fill(f)
                emit_pv(h, kb, exl, at_ps_all[h])
                for qc in range(NQC):
                    if kb == 4 * qc + 3:
                        emit_tail(h, qc, at_ps_all[h][qc])
                if kb == NKB - 1:
                    nc.gpsimd.collective_compute(
                        "AllToAll",
                        mybir.AluOpType.bypass,
                        ins=[a2a_in[h][:].opt()],
                        outs=[a2a_out[h][:].opt()],
                        replica_groups=[list(range(8))],
                    )
                    emit_lh_loads(h)

            # ---- tail: h1's then h2's proj slices cover the h2/h3 A2A
            # windows, then h3's slices stream the output DMAs per (st, nn2)
            # half-row so the final HBM writes overlap the matmuls ----
            for j in range(8):
                emit_proj_group(1, j // 2, j % 2)
            for j in range(8):
                emit_proj_group(2, j // 2, j % 2)
            for st in range(4):
                for nn2 in range(2):
                    emit_proj_group(3, st, nn2, dma_out=True)

    nc.compile()
    return nc


def _get_compiled():
    global _COMPILED
    if _COMPILED is None:
        _COMPILED = _build()
    return _COMPILED


def make_in_maps(x, attention_mask, w_attn, b_attn, w_proj, b_proj):
    x = np.asarray(x, dtype=np.float32)
    w_attn = np.asarray(w_attn, dtype=np.float32)
    b_attn = np.asarray(b_attn, dtype=np.float32)
    w_proj = np.asarray(w_proj, dtype=np.float32)
    b_proj = np.asarray(b_proj, dtype=np.float32)

    ki, qi = np.meshgrid(np.arange(P), np.arange(P), indexing="ij")
    causalT = np.where(ki > qi, np.float32(0.0), np.float32(1.0))
    # xT [NX, S] -> e-major [NE, P, S], split [:, :, :SC] / [:, :, SC:]
    x8 = [
        np.ascontiguousarray(x[b].T.astype(BF16).reshape(NE, P, S)) for b in range(B)
    ]
    bv_full = b_attn[2 * NX : 3 * NX].astype(np.float64)
    bp_eff = (b_proj.astype(np.float64) + bv_full @ w_proj.astype(np.float64)).astype(
        np.float32
    )
    bp_row32 = np.ascontiguousarray(bp_eff.reshape(1, NX))

    in_maps = []
    for c in range(8):
        b, g = divmod(c, 4)
        cols = slice(HDW * g, HDW * (g + 1))
        kcols = slice(NX + HDW * g, NX + HDW * (g + 1))
        vcols = slice(2 * NX + HDW * g, 2 * NX + HDW * (g + 1))
        bqk_arr = np.concatenate([b_attn[cols] * 0.125, b_attn[kcols]]).reshape(4, P)
        # fi-major q/k weights; the 1/8 score scale is folded into the q
        # columns (exact in bf16: pure exponent shift)
        wqk = np.concatenate(
            [w_attn[:, cols] * 0.125, w_attn[:, kcols]], axis=1
        )  # [NX, 512]
        wqkf = np.ascontiguousarray(
            wqk.reshape(NE, P, 4, P).transpose(2, 1, 0, 3).reshape(4, P, NX)
        ).astype(BF16)
        wvc = np.ascontiguousarray(
            w_attn[:, vcols].reshape(NE, P, HDW).transpose(1, 0, 2).reshape(P, NE * HDW)
        ).astype(BF16)
        # own-batch proj tiles (h, tp): rows 0:64 = in-batch sender 2tp's
        # head-h w_proj rows, 64:128 = sender 2tp+1's
        wtiles = np.zeros((HL, 2, P, NX), dtype=np.float32)
        for h in range(HL):
            for tp in range(2):
                for half, j in ((0, 2 * tp), (1, 2 * tp + 1)):
                    rows = w_proj[HDW * j + D * h : HDW * j + D * (h + 1), :]
                    wtiles[h, tp, 64 * half : 64 * half + D, :] = rows
        wpc = np.ascontiguousarray(
            wtiles.reshape(2 * HL, P, NX).transpose(1, 0, 2).reshape(P, 2 * HL * NX)
        ).astype(BF16)
        in_maps.append(
            {
                "x0": np.ascontiguousarray(x8[b][:, :, :SC]),
                "xr": np.ascontiguousarray(x8[b][:, :, SC:]),
                "wqkf": wqkf,
                "wvc": wvc,
                "wpc": wpc,
                "bqk": np.ascontiguousarray(bqk_arr),
                "bp32": bp_row32,
                "causb": causalT.astype(BF16),
                "onesb": np.ones((P, 4), dtype=BF16),
                "slotb": np.array(
                    [[1 - b, b, 0, 0]], dtype=np.int32
                ),  # [is_batch0, is_batch1]
            }
        )
    return in_maps


def assemble_out(results):
    out = np.empty((B, S, NX), dtype=np.float32)
    for c in range(8):
        b, g = divmod(c, 4)
        out[b, g * SC : (g + 1) * SC, :] = results[c]["out"]
    return out


def run(in_maps, trace=False):
    from concourse.bass_utils import run_bass_kernel_spmd

    nc = _get_compiled()
    return run_bass_kernel_spmd(nc, in_maps, core_ids=list(range(8)), trace=trace)


def kernel(**inputs) -> np.ndarray:
    in_maps = make_in_maps(**inputs)
    res = run(in_maps)
    return assemble_out(res.results)


if __name__ == "__main__":
    _get_compiled()
    print("build+compile OK")


# revision 19
# speedup vs baseline: 1.3698x; 1.0556x over previous
"""Distributed causal multi-head attention for Trainium2 (8 NeuronCores).

Problem: B=2, S=2048, NX=1024, H=16 heads, D=64.
  qkv = x @ w_attn + b_attn ; q,k,v split; causal softmax(q k^T / 8) v ; @ w_proj + b_proj

Sharding: core c -> batch b=c//4 (data parallel), head group g=c%4 (tensor
parallel, 4 heads). Column-split c_attn; after attention four per-head
AllToAlls reshard heads->sequence so each core computes c_proj for its own
512 output rows with the full hidden dim - no cross-core reduction.

Schedule (v2): phase 1 computes only h0/h1's q,k (fi0/fi2) plus most of v;
h2/h3's q,k chains are woven into h0's attention as PE filler. Attention
runs a global 1-block software pipeline: the score matmuls for block
(h,kb+1) are emitted BEFORE the PV of block (h,kb), so ScalarE's exp of
block kb runs concurrently with the next block's score matmuls and the PE
never stalls on exp. All PSUM->SBUF copies during the attention window run
on DVE (tensor_scalar_add / tensor_copy), keeping ScalarE exp-only; the
1/sqrt(D)=1/8 score scale is folded into the q weights host-side (exact in
bf16 - exponent shift). c_proj slices for head h are woven into head h+1's
blocks; all 8 of h2's slices are held back to cover the last A2A's
sync+transfer window, and h3's slices stream per-(st,nn2) output DMAs so
the final HBM writes overlap the remaining matmuls.
"""

import sys

sys.path.insert(0, "/opt/trn_rl_repo")

import numpy as np
import ml_dtypes

BF16 = ml_dtypes.bfloat16

B = 2
S = 2048
NX = 1024
H = 16
D = 64
G = 4            # head groups (tensor-parallel)
HL = H // G      # heads per core = 4
HDW = HL * D     # head-group width = 256
P = 128
SC = 512         # output chunk (A2A granularity)
NQC = S // SC    # 4 chunks
NE = NX // P     # 8 contraction tiles
NKB = S // P     # 16 key blocks
WQ = 1024        # max score-tile width

_COMPILED = None


def _build():
    import concourse.bass as bass  # noqa: F401
    from concourse.bass import ds
    import concourse.mybir as mybir
    import concourse.tile as tile
    from concourse import bacc

    f32 = mybir.dt.float32
    i32 = mybir.dt.int32
    bf16 = mybir.dt.bfloat16
    Identity = mybir.ActivationFunctionType.Identity
    Exp = mybir.ActivationFunctionType.Exp

    nc = bacc.Bacc("TRN2", target_bir_lowering=False, debug=False, num_devices=8)

    x0 = nc.dram_tensor("x0", [NE, P, SC], bf16, kind="ExternalInput")
    xr = nc.dram_tensor("xr", [NE, P, S - SC], bf16, kind="ExternalInput")
    wqkf = nc.dram_tensor("wqkf", [4, P, NX], bf16, kind="ExternalInput")
    wvc = nc.dram_tensor("wvc", [P, NE * HDW], bf16, kind="ExternalInput")
    wpc = nc.dram_tensor("wpc", [P, 2 * HL * NX], bf16, kind="ExternalInput")
    bqk = nc.dram_tensor("bqk", [4, P], f32, kind="ExternalInput")
    bp32 = nc.dram_tensor("bp32", [1, NX], f32, kind="ExternalInput")
    onesb = nc.dram_tensor("onesb", [P, 4], bf16, kind="ExternalInput")
    causb = nc.dram_tensor("causb", [P, P], bf16, kind="ExternalInput")
    slotb = nc.dram_tensor("slotb", [1, 4], i32, kind="ExternalInput")
    out_ext = nc.dram_tensor("out", [SC, NX], f32, kind="ExternalOutput")

    with tile.TileContext(nc) as tc:
        with (
            tc.tile_pool(name="const", bufs=1) as const_pool,
            tc.tile_pool(name="xt", bufs=1) as xt_pool,
            tc.tile_pool(name="w", bufs=1) as w_pool,
            tc.tile_pool(name="qkt", bufs=1) as qkt_pool,
            tc.tile_pool(name="vsb", bufs=1) as v_pool,
            tc.tile_pool(name="lh", bufs=1) as lh_pool,
            tc.tile_pool(name="exp", bufs=12) as exp_pool,
            tc.tile_pool(name="osb", bufs=1) as osb_pool,
            tc.tile_pool(name="small", bufs=3) as small_pool,
            tc.tile_pool(name="wide", bufs=1, space="PSUM") as wide_ps,
            tc.tile_pool(name="scps", bufs=3, space="PSUM") as scps_pool,
            tc.tile_pool(name="atps", bufs=4, space="PSUM") as at_ps_pool,
            tc.tile_pool(name="dram", bufs=1, space="DRAM") as dram_pool,
        ):
            # ---- constants + per-batch predicate registers (1/0) ----
            slot_sb = const_pool.tile([1, 4], i32, name="slot_sb")
            nc.sync.dma_start(slot_sb[:], slotb[:])
            r0reg = nc.sync.alloc_register("isb0reg")
            nc.sync.reg_load(r0reg, slot_sb[0:1, 0:1])
            rb0 = nc.sync.snap(r0reg, donate=True, min_val=0, max_val=1)
            r1reg = nc.sync.alloc_register("isb1reg")
            nc.sync.reg_load(r1reg, slot_sb[0:1, 1:2])
            rb1 = nc.sync.snap(r1reg, donate=True, min_val=0, max_val=1)
            # scalar-engine copies for ScalarE-triggered predicated DMAs
            r0s = nc.scalar.alloc_register("isb0s")
            nc.scalar.reg_load(r0s, slot_sb[0:1, 0:1])
            rb0s = nc.scalar.snap(r0s, donate=True, min_val=0, max_val=1)
            r1s = nc.scalar.alloc_register("isb1s")
            nc.scalar.reg_load(r1s, slot_sb[0:1, 1:2])
            rb1s = nc.scalar.snap(r1s, donate=True, min_val=0, max_val=1)

            bqk_sb = const_pool.tile([P, 4], f32, name="bqk_sb")
            for fi in range(4):
                nc.sync.dma_start(bqk_sb[:, fi : fi + 1], bqk[fi : fi + 1, :])
            bp_sb = const_pool.tile([1, NX], f32, name="bp_sb")
            nc.sync.dma_start(bp_sb[:], bp32[:])
            czb_sb = const_pool.tile([P, P], bf16, name="czb_sb")
            nc.sync.dma_start(czb_sb[:], causb[:])

            # ---- dummy A2A fired at kernel start: absorbs the cores'
            # launch stagger and the collective stream's one-time setup cost
            # while phase 1 computes, so A2A(h0) is never queued behind it ----
            a2ad_in = dram_pool.tile([8, 1, P], bf16, name="a2ad_in")
            a2ad_out = dram_pool.tile([8, 1, P], bf16, name="a2ad_out")
            for s in range(8):
                nc.sync.dma_start(a2ad_in[s], czb_sb[0:1, 0:P])
            nc.gpsimd.collective_compute(
                "AllToAll",
                mybir.AluOpType.bypass,
                ins=[a2ad_in[:].opt()],
                outs=[a2ad_out[:].opt()],
                replica_groups=[list(range(8))],
            )

            # ---- weight + x loads: first-needed first ----
            wqk_sb = {}
            for fi in range(4):
                wqk_sb[fi] = w_pool.tile([P, NX], bf16, name=f"wqk_sb{fi}")
            # wqk0 in per-e pieces: the first matmul's weight dep is 32KB on
            # its own DMA queue instead of a 256KB monolith sharing bandwidth
            for e in range(NE):
                nc.sync.dma_start(
                    wqk_sb[0][:, e * P : (e + 1) * P], wqkf[0][:, e * P : (e + 1) * P]
                )
            xt0_sb = {}
            wv_sb = {}
            for e in range(NE):
                t = xt_pool.tile([P, SC], bf16, name=f"xt0_{e}")
                nc.sync.dma_start(t[:], x0[e])
                xt0_sb[e] = t
                tv = w_pool.tile([P, HDW], bf16, name=f"wv_sb{e}")
                nc.sync.dma_start(tv[:], wvc[:, e * HDW : (e + 1) * HDW])
                wv_sb[e] = tv
                if e == 3:
                    nc.sync.dma_start(wqk_sb[2][:], wqkf[2])
            xtr_sb = {}
            for e in range(NE):
                t = xt_pool.tile([P, S - SC], bf16, name=f"xtr_{e}")
                nc.sync.dma_start(t[:], xr[e])
                xtr_sb[e] = t
            nc.sync.dma_start(wqk_sb[1][:], wqkf[1])
            nc.sync.dma_start(wqk_sb[3][:], wqkf[3])
            wp_sb = w_pool.tile([P, 2 * HL * NX], bf16, name="wp_sb")
            nc.sync.dma_start(wp_sb[:], wpc[:])

            def xt_slice(e, c0, w):
                if c0 < SC:
                    return xt0_sb[e][:, c0 : c0 + w]
                return xtr_sb[e][:, c0 - SC : c0 - SC + w]

            # ---- persistent q/k/v SBUF state ----
            qkt_sb = {}
            for fi in range(2):
                qkt_sb[fi] = qkt_pool.tile(
                    [P, S], bf16, name=f"qkt{fi}", tag=f"qktw{fi}"
                )
            # per-head kT with the other head's rows zeroed: score matmuls
            # run at K=128 (zeros annihilate the foreign q rows), keeping
            # switching activity low for the HAM power governor
            ktz_sb = {}
            for h in range(HL):
                ktz_sb[h] = qkt_pool.tile([P, S], bf16, name=f"ktz{h}", tag=f"ktz{h}")
                nc.gpsimd.memset(ktz_sb[h][:], 0.0)
            v_sb = {}

            # ---- chain emitters (qk / v); copies on ACT in phase 1,
            # DVE during the attention window so ScalarE stays exp-only ----
            def emit_qk_chain(fi, sc, on_act):
                ps = at_ps_pool.tile(
                    [P, SC], f32, tag="atps", name=f"qk_ps{fi}_{sc}"
                )
                for e in range(NE):
                    nc.tensor.matmul(
                        ps[:],
                        wqk_sb[fi][:, e * P : (e + 1) * P],
                        xt_slice(e, sc * SC, SC),
                        start=(e == 0),
                        stop=(e == NE - 1),
                    )
                cols = slice(sc * SC, (sc + 1) * SC)
                if fi < 2:
                    if on_act:
                        nc.scalar.activation(
                            qkt_sb[fi][:, cols], ps[:], Identity,
                            bias=bqk_sb[:, fi : fi + 1],
                        )
                    else:
                        nc.vector.tensor_scalar_add(
                            qkt_sb[fi][:, cols], ps[:], bqk_sb[:, fi : fi + 1]
                        )
                else:
                    for hh in range(2):
                        h = 2 * (fi - 2) + hh
                        r0 = 64 * hh
                        if on_act:
                            nc.scalar.activation(
                                ktz_sb[h][r0 : r0 + D, cols],
                                ps[r0 : r0 + D, :],
                                Identity,
                                bias=bqk_sb[r0 : r0 + D, fi : fi + 1],
                            )
                        else:
                            nc.vector.tensor_scalar_add(
                                ktz_sb[h][r0 : r0 + D, cols],
                                ps[r0 : r0 + D, :],
                                bqk_sb[r0 : r0 + D, fi : fi + 1],
                            )

            def emit_v_chain(si, on_act):
                sc, j = divmod(si, 4)
                psv = wide_ps.tile([P, HDW], f32, tag="wide", name=f"v_ps{si}")
                for e in range(NE):
                    nc.tensor.matmul(
                        psv[:],
                        xt_slice(e, sc * SC + j * P, P),
                        wv_sb[e][:],
                        start=(e == 0),
                        stop=(e == NE - 1),
                    )
                # per-head 128-wide slots: [v(64) | ones(1) | zeros(63)]
                vt = v_pool.tile([P, HL * P], bf16, name=f"v{si}")
                nc.gpsimd.memset(vt[:], 0.0)
                nc.sync.dma_start(
                    vt[:].rearrange("p (h u) -> p h u", h=HL)[:, :, D : D + 1],
                    onesb[:],
                )
                dst = vt[:].rearrange("p (h u) -> p h u", h=HL)[:, :, 0:D]
                src = psv[:].rearrange("p (h u) -> p h u", h=HL)
                if on_act:
                    nc.scalar.activation(dst, src, Identity)
                else:
                    nc.vector.tensor_copy(dst, src)
                v_sb[si] = vt

            # ---- phase 1a: fi0/fi2 q,k + v for token blocks 0..11;
            # sc0 first (x0-only work) so the xtr DMAs can land ----
            emit_qk_chain(0, 0, True)
            emit_v_chain(0, True)
            emit_qk_chain(2, 0, True)
            emit_v_chain(1, True)
            emit_v_chain(2, True)
            emit_v_chain(3, True)
            emit_qk_chain(0, 1, True)
            emit_v_chain(4, True)
            emit_qk_chain(2, 1, True)
            emit_v_chain(5, True)
            emit_v_chain(6, True)
            emit_v_chain(7, True)
            emit_qk_chain(0, 2, True)
            emit_v_chain(8, True)
            emit_qk_chain(2, 2, True)
            emit_v_chain(9, True)
            emit_v_chain(10, True)
            emit_v_chain(11, True)
            emit_qk_chain(0, 3, True)
            emit_qk_chain(2, 3, True)

            # ---- A2A buffers ----
            a2a_in = {}
            a2a_out = {}
            for h in range(HL):
                a2a_in[h] = dram_pool.tile([8, D, SC], bf16, name=f"a2a_in{h}")
                a2a_out[h] = dram_pool.tile([8, D, SC], bf16, name=f"a2a_out{h}")

            def emit_tail(h, qc, at_ps):
                # softmax denominator comes from the ones-column of v; the
                # v-bias is folded into bp on the host
                dn32 = small_pool.tile([1, SC], f32, tag="dn32", name=f"dn32{qc}_{h}")
                nc.vector.tensor_copy(dn32[:], at_ps[D : D + 1, :])
                rc32 = small_pool.tile([1, SC], f32, tag="rc32", name=f"rc32{qc}_{h}")
                nc.vector.reciprocal_approx_fast(rc32[:], dn32[:])
                rb = small_pool.tile([D, SC], f32, tag="rbsb", name=f"rbsb{qc}_{h}")
                nc.gpsimd.partition_broadcast(rb[:], rc32[:])
                ath = small_pool.tile(
                    [D, SC], bf16, tag="ath", bufs=3, name=f"ath{qc}_{h}"
                )
                nc.vector.tensor_mul(ath[:], at_ps[0:D, :], rb[:])
                nc.sync.dma_start(a2a_in[h][qc, :, :], ath[:])
                nc.sync.dma_start(a2a_in[h][qc + 4, :, :], ath[:])

            def emit_score_exp(h, kb):
                # score tiles (transposed [k, q]) + exp for key block kb;
                # 512-wide pieces through the dedicated 3-deep scps pool so
                # the next block's score matmuls never serialize on this
                # block's exp
                fi_q = h // 2
                q0 = P * kb
                out = []
                for s2 in range((S - q0 + SC - 1) // SC):
                    w0 = q0 + s2 * SC
                    ww = min(SC, S - w0)
                    scp = scps_pool.tile(
                        [P, SC], f32, tag="scps", name=f"sc{h}_{kb}_{s2}"
                    )
                    nc.tensor.matmul(
                        scp[:, 0:ww],
                        ktz_sb[h][:, q0 : q0 + P],
                        qkt_sb[fi_q][:, w0 : w0 + ww],
                        start=True,
                        stop=True,
                    )
                    ex = exp_pool.tile(
                        [P, SC], bf16, tag="exp", name=f"ex{h}_{kb}_{s2}"
                    )
                    nc.scalar.activation(ex[:, 0:ww], scp[:, 0:ww], Exp)
                    if s2 == 0:
                        # diagonal block: zero the non-causal upper triangle
                        nc.vector.tensor_mul(ex[:, 0:P], ex[:, 0:P], czb_sb[:])
                    out.append((ex, w0, ww))
                return out

            def emit_pv(h, kb, exl, at_ps):
                for ex, w0, ww in exl:
                    qc_lo = w0 // SC
                    qc_hi = (w0 + ww - 1) // SC
                    for qc in range(qc_lo, qc_hi + 1):
                        a0 = max(w0, qc * SC)
                        a1 = min(w0 + ww, (qc + 1) * SC)
                        nc.tensor.matmul(
                            at_ps[qc][:, a0 - qc * SC : a1 - qc * SC],
                            v_sb[kb][:, h * P : (h + 1) * P],
                            ex[:, a0 - w0 : a1 - w0],
                            start=(kb == 0),
                            stop=(kb == 4 * qc + 3),
                        )

            # ---- c_proj state + emitters ----
            osb = {}
            for st in range(4):
                osb[st] = osb_pool.tile([P, NX], f32, name=f"osb{st}")
                nc.gpsimd.partition_broadcast(osb[st][:], bp_sb[:])
            lh = {}

            def emit_lh_loads(h):
                # predicated static loads straight from a2a_out: the own-batch
                # slot pair (4b+2tp, 4b+2tp+1) is 128KB contiguous; emit both
                # batch variants, exactly one runs (cond register), killing the
                # dynamic DRAM->DRAM slot-select copy and its latency. For the
                # last head the triggers split across SP and ScalarE (idle by
                # then) so the serial trigger chain halves.
                for tp in range(2):
                    t = lh_pool.tile([P, SC], bf16, name=f"lh{h}_{tp}")
                    on_act = h == HL - 1 and tp == 1
                    eng = nc.scalar if on_act else nc.sync
                    conds = ((0, rb0s), (1, rb1s)) if on_act else ((0, rb0), (1, rb1))
                    for bvar, rb in conds:
                        src = a2a_out[h][4 * bvar + 2 * tp : 4 * bvar + 2 * tp + 2]
                        eng.dma_start(
                            t[:], src.rearrange("a d c -> (a d) c"), cond=rb
                        )
                    lh[h, tp] = t

            def emit_proj_group(h, st, nn2, dma_out=False):
                pp = wide_ps.tile([P, SC], f32, tag="wide", name=f"pj{h}_{st}_{nn2}")
                for tp in range(2):
                    wcol = (2 * h + tp) * NX + nn2 * SC
                    nc.tensor.matmul(
                        pp[:],
                        lh[h, tp][:, st * P : (st + 1) * P],
                        wp_sb[:, wcol : wcol + SC],
                        start=(tp == 0),
                        stop=(tp == 1),
                    )
                dst = osb[st][:, nn2 * SC : (nn2 + 1) * SC]
                nc.vector.tensor_add(dst, dst, pp[:])
                if dma_out:
                    # alternate trigger engines so the 8 output writes don't
                    # serialize on one sequencer's ~0.6us-per-trigger chain
                    eng = nc.sync if (st + nn2) % 2 == 0 else nc.scalar
                    eng.dma_start(
                        out_ext[st * P : (st + 1) * P, nn2 * SC : (nn2 + 1) * SC],
                        dst,
                    )

            # ---- fill schedule: work woven between score(kb+1) and pv(kb).
            # h0: remaining v chains + h2/h3's q,k chains (copies on DVE);
            # h1: h0's proj slices; h2: h1's; h3: lean (h2's slices are the
            # reserve that covers the final A2A window).
            fills = {}
            for j in range(4):
                fills[0, j] = ("v", 12 + j)
            for sc in range(4):
                fills[0, 4 + sc] = ("qk", 1, sc)
            for sc in range(4):
                fills[0, 8 + sc] = ("qk", 3, sc)
            for j in range(4):
                fills[1, 4 + j] = ("proj", 0, j // 2, j % 2)
            for j in range(4):
                fills[3, 2 + 2 * j] = ("proj", 0, 2 + j // 2, j % 2)

            def emit_fill(f):
                if f[0] == "v":
                    emit_v_chain(f[1], on_act=False)
                elif f[0] == "qk":
                    emit_qk_chain(f[1], f[2], on_act=False)
                else:
                    emit_proj_group(f[1], f[2], f[3])

            # ---- attention: global 1-block software pipeline ----
            blocks = [(h, kb) for h in range(HL) for kb in range(NKB)]
            at_ps_all = {}
            for h in range(HL):
                at_ps_all[h] = {}
            exl_next = emit_score_exp(0, 0)
            for i, (h, kb) in enumerate(blocks):
                if kb == 0:
                    for qc in range(NQC):
                        at_ps_all[h][qc] = at_ps_pool.tile(
                            [P, SC], f32, tag="atps", name=f"at_ps{qc}_{h}"
                        )
                exl = exl_next
                if i + 1 < len(blocks):
                    exl_next = emit_score_exp(*blocks[i + 1])
                f = fills.get((h, kb))
                if f is not None:
                    emit_fill(f)
                emit_pv(h, kb, exl, at_ps_all[h])
                for qc in range(NQC):
                    if kb == 4 * qc + 3:
                        emit_tail(h, qc, at_ps_all[h][qc])
                if kb == NKB - 1:
                    nc.gpsimd.collective_compute(
                        "AllToAll",
                        mybir.AluOpType.bypass,
                        ins=[a2a_in[h][:].opt()],
                        outs=[a2a_out[h][:].opt()],
                        replica_groups=[list(range(8))],
                    )
                    emit_lh_loads(h)

            # ---- tail: h1's then h2's proj slices cover the h2/h3 A2A
            # windows, then h3's slices stream the output DMAs per (st, nn2)
            # half-row so the final HBM writes overlap the matmuls ----
            for j in range(8):
                emit_proj_group(1, j // 2, j % 2)
            for j in range(8):
                emit_proj_group(2, j // 2, j % 2)
            for st in range(4):
                for nn2 in range(2):
                    emit_proj_group(3, st, nn2, dma_out=True)

    nc.compile()
    return nc


def _get_compiled():
    global _COMPILED
    if _COMPILED is None:
        _COMPILED = _build()
    return _COMPILED


def make_in_maps(x, attention_mask, w_attn, b_attn, w_proj, b_proj):
    x = np.asarray(x, dtype=np.float32)
    w_attn = np.asarray(w_attn, dtype=np.float32)
    b_attn = np.asarray(b_attn, dtype=np.float32)
    w_proj = np.asarray(w_proj, dtype=np.float32)
    b_proj = np.asarray(b_proj, dtype=np.float32)

    ki, qi = np.meshgrid(np.arange(P), np.arange(P), indexing="ij")
    causalT = np.where(ki > qi, np.float32(0.0), np.float32(1.0))
    # xT [NX, S] -> e-major [NE, P, S], split [:, :, :SC] / [:, :, SC:]
    x8 = [
        np.ascontiguousarray(x[b].T.astype(BF16).reshape(NE, P, S)) for b in range(B)
    ]
    bv_full = b_attn[2 * NX : 3 * NX].astype(np.float64)
    bp_eff = (b_proj.astype(np.float64) + bv_full @ w_proj.astype(np.float64)).astype(
        np.float32
    )
    bp_row32 = np.ascontiguousarray(bp_eff.reshape(1, NX))

    in_maps = []
    for c in range(8):
        b, g = divmod(c, 4)
        cols = slice(HDW * g, HDW * (g + 1))
        kcols = slice(NX + HDW * g, NX + HDW * (g + 1))
        vcols = slice(2 * NX + HDW * g, 2 * NX + HDW * (g + 1))
        bqk_arr = np.concatenate([b_attn[cols] * 0.125, b_attn[kcols]]).reshape(4, P)
        # fi-major q/k weights; the 1/8 score scale is folded into the q
        # columns (exact in bf16: pure exponent shift)
        wqk = np.concatenate(
            [w_attn[:, cols] * 0.125, w_attn[:, kcols]], axis=1
        )  # [NX, 512]
        wqkf = np.ascontiguousarray(
            wqk.reshape(NE, P, 4, P).transpose(2, 1, 0, 3).reshape(4, P, NX)
        ).astype(BF16)
        wvc = np.ascontiguousarray(
            w_attn[:, vcols].reshape(NE, P, HDW).transpose(1, 0, 2).reshape(P, NE * HDW)
        ).astype(BF16)
        # own-batch proj tiles (h, tp): rows 0:64 = in-batch sender 2tp's
        # head-h w_proj rows, 64:128 = sender 2tp+1's
        wtiles = np.zeros((HL, 2, P, NX), dtype=np.float32)
        for h in range(HL):
            for tp in range(2):
                for half, j in ((0, 2 * tp), (1, 2 * tp + 1)):
                    rows = w_proj[HDW * j + D * h : HDW * j + D * (h + 1), :]
                    wtiles[h, tp, 64 * half : 64 * half + D, :] = rows
        wpc = np.ascontiguousarray(
            wtiles.reshape(2 * HL, P, NX).transpose(1, 0, 2).reshape(P, 2 * HL * NX)
        ).astype(BF16)
        in_maps.append(
            {
                "x0": np.ascontiguousarray(x8[b][:, :, :SC]),
                "xr": np.ascontiguousarray(x8[b][:, :, SC:]),
                "wqkf": wqkf,
                "wvc": wvc,
                "wpc": wpc,
                "bqk": np.ascontiguousarray(bqk_arr),
                "bp32": bp_row32,
                "causb": causalT.astype(BF16),
                "onesb": np.ones((P, 4), dtype=BF16),
                "slotb": np.array(
                    [[1 - b, b, 0, 0]], dtype=np.int32
                ),  # [is_batch0, is_batch1]
            }
        )
    return in_maps


def assemble_out(results):
    out = np.empty((B, S, NX), dtype=np.float32)
    for c in range(8):
        b, g = divmod(c, 4)
        out[b, g * SC : (g + 1) * SC, :] = results[c]["out"]
    return out


def run(in_maps, trace=False):
    from concourse.bass_utils import run_bass_kernel_spmd

    nc = _get_compiled()
    return run_bass_kernel_spmd(nc, in_maps, core_ids=list(range(8)), trace=trace)


def kernel(**inputs) -> np.ndarray:
    in_maps = make_in_maps(**inputs)
    res = run(in_maps)
    return assemble_out(res.results)


if __name__ == "__main__":
    _get_compiled()
    print("build+compile OK")


# revision 21
# speedup vs baseline: 1.4329x; 1.0460x over previous
"""Distributed causal multi-head attention for Trainium2 (8 NeuronCores).

Problem: B=2, S=2048, NX=1024, H=16 heads, D=64.
  qkv = x @ w_attn + b_attn ; q,k,v split; causal softmax(q k^T / 8) v ; @ w_proj + b_proj

Sharding: core c -> batch b=c//4 (data parallel), head group g=c%4 (tensor
parallel, 4 heads). Column-split c_attn; after attention four per-head
AllToAlls reshard heads->sequence so each core computes c_proj for its own
512 output rows with the full hidden dim - no cross-core reduction.

Schedule (v2): phase 1 computes only h0/h1's q,k (fi0/fi2) plus most of v;
h2/h3's q,k chains are woven into h0's attention as PE filler. Attention
runs a global 1-block software pipeline: the score matmuls for block
(h,kb+1) are emitted BEFORE the PV of block (h,kb), so ScalarE's exp of
block kb runs concurrently with the next block's score matmuls and the PE
never stalls on exp. All PSUM->SBUF copies during the attention window run
on DVE (tensor_scalar_add / tensor_copy), keeping ScalarE exp-only; the
1/sqrt(D)=1/8 score scale is folded into the q weights host-side (exact in
bf16 - exponent shift). c_proj slices for head h are woven into head h+1's
blocks; all 8 of h2's slices are held back to cover the last A2A's
sync+transfer window, and h3's slices stream per-(st,nn2) output DMAs so
the final HBM writes overlap the remaining matmuls.
"""

import sys

sys.path.insert(0, "/opt/trn_rl_repo")

import numpy as np
import ml_dtypes

BF16 = ml_dtypes.bfloat16

B = 2
S = 2048
NX = 1024
H = 16
D = 64
G = 4            # head groups (tensor-parallel)
HL = H // G      # heads per core = 4
HDW = HL * D     # head-group width = 256
P = 128
SC = 512         # output chunk (A2A granularity)
NQC = S // SC    # 4 chunks
NE = NX // P     # 8 contraction tiles
NKB = S // P     # 16 key blocks
WQ = 1024        # max score-tile width

_COMPILED = None


def _build():
    import concourse.bass as bass  # noqa: F401
    from concourse.bass import ds
    import concourse.mybir as mybir
    import concourse.tile as tile
    from concourse import bacc

    f32 = mybir.dt.float32
    i32 = mybir.dt.int32
    bf16 = mybir.dt.bfloat16
    Identity = mybir.ActivationFunctionType.Identity
    Exp = mybir.ActivationFunctionType.Exp

    nc = bacc.Bacc("TRN2", target_bir_lowering=False, debug=False, num_devices=8)

    x0 = nc.dram_tensor("x0", [NE, P, SC], bf16, kind="ExternalInput")
    xr = nc.dram_tensor("xr", [NE, P, S - SC], bf16, kind="ExternalInput")
    wqkf = nc.dram_tensor("wqkf", [4, P, NX], bf16, kind="ExternalInput")
    wvc = nc.dram_tensor("wvc", [P, NE * HDW], bf16, kind="ExternalInput")
    wpc = nc.dram_tensor("wpc", [P, 2 * HL * NX], bf16, kind="ExternalInput")
    bqk = nc.dram_tensor("bqk", [4, P], f32, kind="ExternalInput")
    bp32 = nc.dram_tensor("bp32", [1, NX], f32, kind="ExternalInput")
    onesb = nc.dram_tensor("onesb", [P, 4], bf16, kind="ExternalInput")
    causb = nc.dram_tensor("causb", [P, P], bf16, kind="ExternalInput")
    slotb = nc.dram_tensor("slotb", [1, 4], i32, kind="ExternalInput")
    out_ext = nc.dram_tensor("out", [SC, NX], f32, kind="ExternalOutput")

    with tile.TileContext(nc) as tc:
        with (
            tc.tile_pool(name="const", bufs=1) as const_pool,
            tc.tile_pool(name="xt", bufs=1) as xt_pool,
            tc.tile_pool(name="w", bufs=1) as w_pool,
            tc.tile_pool(name="qkt", bufs=1) as qkt_pool,
            tc.tile_pool(name="vsb", bufs=1) as v_pool,
            tc.tile_pool(name="lh", bufs=1) as lh_pool,
            tc.tile_pool(name="exp", bufs=12) as exp_pool,
            tc.tile_pool(name="osb", bufs=1) as osb_pool,
            tc.tile_pool(name="small", bufs=3) as small_pool,
            tc.tile_pool(name="wide", bufs=1, space="PSUM") as wide_ps,
            tc.tile_pool(name="scps", bufs=3, space="PSUM") as scps_pool,
            tc.tile_pool(name="atps", bufs=4, space="PSUM") as at_ps_pool,
            tc.tile_pool(name="dram", bufs=1, space="DRAM") as dram_pool,
        ):
            # ---- constants + per-batch predicate registers (1/0) ----
            slot_sb = const_pool.tile([1, 4], i32, name="slot_sb")
            nc.sync.dma_start(slot_sb[:], slotb[:])
            r0reg = nc.sync.alloc_register("isb0reg")
            nc.sync.reg_load(r0reg, slot_sb[0:1, 0:1])
            rb0 = nc.sync.snap(r0reg, donate=True, min_val=0, max_val=1)
            r1reg = nc.sync.alloc_register("isb1reg")
            nc.sync.reg_load(r1reg, slot_sb[0:1, 1:2])
            rb1 = nc.sync.snap(r1reg, donate=True, min_val=0, max_val=1)
            # scalar-engine copies for ScalarE-triggered predicated DMAs
            r0s = nc.scalar.alloc_register("isb0s")
            nc.scalar.reg_load(r0s, slot_sb[0:1, 0:1])
            rb0s = nc.scalar.snap(r0s, donate=True, min_val=0, max_val=1)
            r1s = nc.scalar.alloc_register("isb1s")
            nc.scalar.reg_load(r1s, slot_sb[0:1, 1:2])
            rb1s = nc.scalar.snap(r1s, donate=True, min_val=0, max_val=1)

            bqk_sb = const_pool.tile([P, 4], f32, name="bqk_sb")
            for fi in range(4):
                nc.sync.dma_start(bqk_sb[:, fi : fi + 1], bqk[fi : fi + 1, :])
            bp_sb = const_pool.tile([1, NX], f32, name="bp_sb")
            nc.sync.dma_start(bp_sb[:], bp32[:])
            czb_sb = const_pool.tile([P, P], bf16, name="czb_sb")
            nc.sync.dma_start(czb_sb[:], causb[:])

            # ---- dummy A2A fired at kernel start: absorbs the cores'
            # launch stagger and the collective stream's one-time setup cost
            # while phase 1 computes, so A2A(h0) is never queued behind it ----
            a2ad_in = dram_pool.tile([8, 1, P], bf16, name="a2ad_in")
            a2ad_out = dram_pool.tile([8, 1, P], bf16, name="a2ad_out")
            for s in range(8):
                nc.sync.dma_start(a2ad_in[s], czb_sb[0:1, 0:P])
            nc.gpsimd.collective_compute(
                "AllToAll",
                mybir.AluOpType.bypass,
                ins=[a2ad_in[:].opt()],
                outs=[a2ad_out[:].opt()],
                replica_groups=[list(range(8))],
            )

            # ---- weight + x loads: first-needed first ----
            wqk_sb = {}
            for fi in range(4):
                wqk_sb[fi] = w_pool.tile([P, NX], bf16, name=f"wqk_sb{fi}")
            # wqk0 in per-e pieces: the first matmul's weight dep is 32KB on
            # its own DMA queue instead of a 256KB monolith sharing bandwidth
            for e in range(NE):
                nc.sync.dma_start(
                    wqk_sb[0][:, e * P : (e + 1) * P], wqkf[0][:, e * P : (e + 1) * P]
                )
            xt0_sb = {}
            wv_sb = {}
            for e in range(NE):
                t = xt_pool.tile([P, SC], bf16, name=f"xt0_{e}")
                nc.sync.dma_start(t[:], x0[e])
                xt0_sb[e] = t
                tv = w_pool.tile([P, HDW], bf16, name=f"wv_sb{e}")
                nc.sync.dma_start(tv[:], wvc[:, e * HDW : (e + 1) * HDW])
                wv_sb[e] = tv
                if e == 3:
                    nc.sync.dma_start(wqk_sb[2][:], wqkf[2])
            xtr_sb = {}
            for e in range(NE):
                t = xt_pool.tile([P, S - SC], bf16, name=f"xtr_{e}")
                nc.sync.dma_start(t[:], xr[e])
                xtr_sb[e] = t
            nc.sync.dma_start(wqk_sb[1][:], wqkf[1])
            nc.sync.dma_start(wqk_sb[3][:], wqkf[3])
            wp_sb = w_pool.tile([P, 2 * HL * NX], bf16, name="wp_sb")
            nc.sync.dma_start(wp_sb[:], wpc[:])

            def xt_slice(e, c0, w):
                if c0 < SC:
                    return xt0_sb[e][:, c0 : c0 + w]
                return xtr_sb[e][:, c0 - SC : c0 - SC + w]

            # ---- persistent q/k/v SBUF state ----
            qkt_sb = {}
            for fi in range(2):
                qkt_sb[fi] = qkt_pool.tile(
                    [P, S], bf16, name=f"qkt{fi}", tag=f"qktw{fi}"
                )
            # per-head kT with the other head's rows zeroed: score matmuls
            # run at K=128 (zeros annihilate the foreign q rows), keeping
            # switching activity low for the HAM power governor
            ktz_sb = {}
            for h in range(HL):
                ktz_sb[h] = qkt_pool.tile([P, S], bf16, name=f"ktz{h}", tag=f"ktz{h}")
                nc.gpsimd.memset(ktz_sb[h][:], 0.0)
            v_sb = {}

            # ---- chain emitters (qk / v); copies on ACT in phase 1,
            # DVE during the attention window so ScalarE stays exp-only ----
            def emit_qk_chain(fi, sc, on_act):
                ps = at_ps_pool.tile(
                    [P, SC], f32, tag="atps", name=f"qk_ps{fi}_{sc}"
                )
                for e in range(NE):
                    nc.tensor.matmul(
                        ps[:],
                        wqk_sb[fi][:, e * P : (e + 1) * P],
                        xt_slice(e, sc * SC, SC),
                        start=(e == 0),
                        stop=(e == NE - 1),
                    )
                cols = slice(sc * SC, (sc + 1) * SC)
                if fi < 2:
                    if on_act:
                        nc.scalar.activation(
                            qkt_sb[fi][:, cols], ps[:], Identity,
                            bias=bqk_sb[:, fi : fi + 1],
                        )
                    else:
                        nc.vector.tensor_scalar_add(
                            qkt_sb[fi][:, cols], ps[:], bqk_sb[:, fi : fi + 1]
                        )
                else:
                    for hh in range(2):
                        h = 2 * (fi - 2) + hh
                        r0 = 64 * hh
                        if on_act:
                            nc.scalar.activation(
                                ktz_sb[h][r0 : r0 + D, cols],
                                ps[r0 : r0 + D, :],
                                Identity,
                                bias=bqk_sb[r0 : r0 + D, fi : fi + 1],
                            )
                        else:
                            nc.vector.tensor_scalar_add(
                                ktz_sb[h][r0 : r0 + D, cols],
                                ps[r0 : r0 + D, :],
                                bqk_sb[r0 : r0 + D, fi : fi + 1],
                            )

            def emit_v_chain(si, on_act):
                sc, j = divmod(si, 4)
                psv = wide_ps.tile([P, HDW], f32, tag="wide", name=f"v_ps{si}")
                for e in range(NE):
                    nc.tensor.matmul(
                        psv[:],
                        xt_slice(e, sc * SC + j * P, P),
                        wv_sb[e][:],
                        start=(e == 0),
                        stop=(e == NE - 1),
                    )
                # per-head 128-wide slots: [v(64) | ones(1) | zeros(63)]
                vt = v_pool.tile([P, HL * P], bf16, name=f"v{si}")
                nc.gpsimd.memset(vt[:], 0.0)
                nc.sync.dma_start(
                    vt[:].rearrange("p (h u) -> p h u", h=HL)[:, :, D : D + 1],
                    onesb[:],
                )
                dst = vt[:].rearrange("p (h u) -> p h u", h=HL)[:, :, 0:D]
                src = psv[:].rearrange("p (h u) -> p h u", h=HL)
                if on_act:
                    nc.scalar.activation(dst, src, Identity)
                else:
                    nc.vector.tensor_copy(dst, src)
                v_sb[si] = vt

            # ---- phase 1a: fi0/fi2 q,k + v for token blocks 0..11;
            # sc0 first (x0-only work) so the xtr DMAs can land ----
            emit_qk_chain(0, 0, True)
            emit_v_chain(0, True)
            emit_qk_chain(2, 0, True)
            emit_v_chain(1, True)
            emit_v_chain(2, True)
            emit_v_chain(3, True)
            emit_qk_chain(0, 1, True)
            emit_v_chain(4, True)
            emit_qk_chain(2, 1, True)
            emit_v_chain(5, True)
            emit_v_chain(6, True)
            emit_v_chain(7, True)
            emit_qk_chain(0, 2, True)
            emit_v_chain(8, True)
            emit_qk_chain(2, 2, True)
            emit_v_chain(9, True)
            emit_v_chain(10, True)
            emit_v_chain(11, True)
            emit_qk_chain(0, 3, True)
            emit_qk_chain(2, 3, True)

            # ---- A2A buffers ----
            a2a_in = {}
            a2a_out = {}
            for h in range(HL):
                a2a_in[h] = dram_pool.tile([8, D, SC], bf16, name=f"a2a_in{h}")
                a2a_out[h] = dram_pool.tile([8, D, SC], bf16, name=f"a2a_out{h}")

            def emit_tail(h, qc, at_ps):
                # softmax denominator comes from the ones-column of v; the
                # v-bias is folded into bp on the host
                dn32 = small_pool.tile([1, SC], f32, tag="dn32", name=f"dn32{qc}_{h}")
                nc.vector.tensor_copy(dn32[:], at_ps[D : D + 1, :])
                rc32 = small_pool.tile([1, SC], f32, tag="rc32", name=f"rc32{qc}_{h}")
                nc.vector.reciprocal_approx_fast(rc32[:], dn32[:])
                rb = small_pool.tile([D, SC], f32, tag="rbsb", name=f"rbsb{qc}_{h}")
                nc.gpsimd.partition_broadcast(rb[:], rc32[:])
                ath = small_pool.tile(
                    [D, SC], bf16, tag="ath", bufs=3, name=f"ath{qc}_{h}"
                )
                nc.vector.tensor_mul(ath[:], at_ps[0:D, :], rb[:])
                nc.sync.dma_start(a2a_in[h][qc, :, :], ath[:])
                nc.sync.dma_start(a2a_in[h][qc + 4, :, :], ath[:])

            def emit_score_exp(h, kb):
                # score tiles (transposed [k, q]) + exp for key block kb;
                # 512-wide pieces through the dedicated 3-deep scps pool so
                # the next block's score matmuls never serialize on this
                # block's exp
                fi_q = h // 2
                q0 = P * kb
                out = []
                for s2 in range((S - q0 + SC - 1) // SC):
                    w0 = q0 + s2 * SC
                    ww = min(SC, S - w0)
                    scp = scps_pool.tile(
                        [P, SC], f32, tag="scps", name=f"sc{h}_{kb}_{s2}"
                    )
                    nc.tensor.matmul(
                        scp[:, 0:ww],
                        ktz_sb[h][:, q0 : q0 + P],
                        qkt_sb[fi_q][:, w0 : w0 + ww],
                        start=True,
                        stop=True,
                    )
                    ex = exp_pool.tile(
                        [P, SC], bf16, tag="exp", name=f"ex{h}_{kb}_{s2}"
                    )
                    nc.scalar.activation(ex[:, 0:ww], scp[:, 0:ww], Exp)
                    if s2 == 0:
                        # diagonal block: zero the non-causal upper triangle
                        nc.vector.tensor_mul(ex[:, 0:P], ex[:, 0:P], czb_sb[:])
                    out.append((ex, w0, ww))
                return out

            def emit_pv(h, kb, exl, at_ps):
                for ex, w0, ww in exl:
                    qc_lo = w0 // SC
                    qc_hi = (w0 + ww - 1) // SC
                    for qc in range(qc_lo, qc_hi + 1):
                        a0 = max(w0, qc * SC)
                        a1 = min(w0 + ww, (qc + 1) * SC)
                        nc.tensor.matmul(
                            at_ps[qc][:, a0 - qc * SC : a1 - qc * SC],
                            v_sb[kb][:, h * P : (h + 1) * P],
                            ex[:, a0 - w0 : a1 - w0],
                            start=(kb == 0),
                            stop=(kb == 4 * qc + 3),
                        )

            # ---- c_proj state + emitters ----
            osb = {}
            for st in range(4):
                osb[st] = osb_pool.tile([P, NX], f32, name=f"osb{st}")
                nc.gpsimd.partition_broadcast(osb[st][:], bp_sb[:])
            lh = {}

            def emit_lh_loads(h):
                # predicated static loads straight from a2a_out: the own-batch
                # slot pair (4b+2tp, 4b+2tp+1) is 128KB contiguous; emit both
                # batch variants, exactly one runs (cond register), killing the
                # dynamic DRAM->DRAM slot-select copy and its latency. For the
                # last head the triggers split across SP and ScalarE (idle by
                # then) so the serial trigger chain halves.
                for tp in range(2):
                    t = lh_pool.tile([P, SC], bf16, name=f"lh{h}_{tp}")
                    on_act = h == HL - 1 and tp == 1
                    eng = nc.scalar if on_act else nc.sync
                    conds = ((0, rb0s), (1, rb1s)) if on_act else ((0, rb0), (1, rb1))
                    for bvar, rb in conds:
                        src = a2a_out[h][4 * bvar + 2 * tp : 4 * bvar + 2 * tp + 2]
                        eng.dma_start(
                            t[:], src.rearrange("a d c -> (a d) c"), cond=rb
                        )
                    lh[h, tp] = t

            def emit_proj_group(h, st, nn2, dma_out=False):
                pp = wide_ps.tile([P, SC], f32, tag="wide", name=f"pj{h}_{st}_{nn2}")
                for tp in range(2):
                    wcol = (2 * h + tp) * NX + nn2 * SC
                    nc.tensor.matmul(
                        pp[:],
                        lh[h, tp][:, st * P : (st + 1) * P],
                        wp_sb[:, wcol : wcol + SC],
                        start=(tp == 0),
                        stop=(tp == 1),
                    )
                dst = osb[st][:, nn2 * SC : (nn2 + 1) * SC]
                nc.vector.tensor_add(dst, dst, pp[:])
                if dma_out:
                    # alternate trigger engines so the 8 output writes don't
                    # serialize on one sequencer's ~0.6us-per-trigger chain
                    eng = nc.sync if (st + nn2) % 2 == 0 else nc.scalar
                    eng.dma_start(
                        out_ext[st * P : (st + 1) * P, nn2 * SC : (nn2 + 1) * SC],
                        dst,
                    )

            # ---- fill schedule: work woven between score(kb+1) and pv(kb).
            # h0: remaining v chains + h2/h3's q,k chains (copies on DVE);
            # h1: h0's proj slices; h2: h1's; h3: lean (h2's slices are the
            # reserve that covers the final A2A window).
            fills = {}
            for j in range(4):
                fills[0, j] = ("v", 12 + j)
            for sc in range(4):
                fills[0, 4 + sc] = ("qk", 1, sc)
            for sc in range(4):
                fills[0, 8 + sc] = ("qk", 3, sc)
            # lh loads woven at kb9 of the next head: their A2A-done waits
            # block the in-order SP sequencer, so they must sit AFTER the
            # ath staging DMAs of chunks 0/1 in SP program order
            fills[1, 9] = ("lh", 0)
            fills[2, 9] = ("lh", 1)
            for j in range(4):
                fills[1, 10 + j] = ("proj", 0, j // 2, j % 2)
            for j in range(4):
                fills[3, 2 + 2 * j] = ("proj", 0, 2 + j // 2, j % 2)

            def emit_fill(f):
                if f[0] == "v":
                    emit_v_chain(f[1], on_act=False)
                elif f[0] == "qk":
                    emit_qk_chain(f[1], f[2], on_act=False)
                elif f[0] == "lh":
                    emit_lh_loads(f[1])
                else:
                    emit_proj_group(f[1], f[2], f[3])

            # ---- attention: global 1-block software pipeline ----
            blocks = [(h, kb) for h in range(HL) for kb in range(NKB)]
            at_ps_all = {}
            for h in range(HL):
                at_ps_all[h] = {}
            exl_next = emit_score_exp(0, 0)
            for i, (h, kb) in enumerate(blocks):
                if kb == 0:
                    for qc in range(NQC):
                        at_ps_all[h][qc] = at_ps_pool.tile(
                            [P, SC], f32, tag="atps", name=f"at_ps{qc}_{h}"
                        )
                exl = exl_next
                if i + 1 < len(blocks):
                    exl_next = emit_score_exp(*blocks[i + 1])
                f = fills.get((h, kb))
                if f is not None:
                    emit_fill(f)
                emit_pv(h, kb, exl, at_ps_all[h])
                for qc in range(NQC):
                    if kb == 4 * qc + 3:
                        emit_tail(h, qc, at_ps_all[h][qc])
                if kb == NKB - 1:
                    nc.gpsimd.collective_compute(
                        "AllToAll",
                        mybir.AluOpType.bypass,
                        ins=[a2a_in[h][:].opt()],
                        outs=[a2a_out[h][:].opt()],
                        replica_groups=[list(range(8))],
                    )

            # ---- tail: h1+h2's proj slices merged per (st, nn2) cover the
            # h3 A2A window (pp tiles come from the now-free 4-deep atps pool
            # so DVE adds pipeline instead of serializing each group); h3's
            # slices then stream the output DMAs per (st, nn2) half-row ----
            emit_lh_loads(2)
            emit_lh_loads(3)

            def emit_proj12(st, nn2):
                pp = at_ps_pool.tile(
                    [P, SC], f32, tag="atps", name=f"pj12_{st}_{nn2}"
                )
                mms = [(hh, tp) for hh in (1, 2) for tp in range(2)]
                for mi, (hh, tp) in enumerate(mms):
                    wcol = (2 * hh + tp) * NX + nn2 * SC
                    nc.tensor.matmul(
                        pp[:],
                        lh[hh, tp][:, st * P : (st + 1) * P],
                        wp_sb[:, wcol : wcol + SC],
                        start=(mi == 0),
                        stop=(mi == len(mms) - 1),
                    )
                dst = osb[st][:, nn2 * SC : (nn2 + 1) * SC]
                nc.vector.tensor_add(dst, dst, pp[:])

            def emit_proj3(st, nn2):
                pp = at_ps_pool.tile(
                    [P, SC], f32, tag="atps", name=f"pj3_{st}_{nn2}"
                )
                for tp in range(2):
                    wcol = (2 * 3 + tp) * NX + nn2 * SC
                    nc.tensor.matmul(
                        pp[:],
                        lh[3, tp][:, st * P : (st + 1) * P],
                        wp_sb[:, wcol : wcol + SC],
                        start=(tp == 0),
                        stop=(tp == 1),
                    )
                dst = osb[st][:, nn2 * SC : (nn2 + 1) * SC]
                nc.vector.tensor_add(dst, dst, pp[:])
                eng = nc.sync if (st + nn2) % 2 == 0 else nc.scalar
                eng.dma_start(
                    out_ext[st * P : (st + 1) * P, nn2 * SC : (nn2 + 1) * SC], dst
                )

            for st in range(4):
                for nn2 in range(2):
                    emit_proj12(st, nn2)
            for st in range(4):
                for nn2 in range(2):
                    emit_proj3(st, nn2)

    nc.compile()
    return nc


def _get_compiled():
    global _COMPILED
    if _COMPILED is None:
        _COMPILED = _build()
    return _COMPILED


def make_in_maps(x, attention_mask, w_attn, b_attn, w_proj, b_proj):
    x = np.asarray(x, dtype=np.float32)
    w_attn = np.asarray(w_attn, dtype=np.float32)
    b_attn = np.asarray(b_attn, dtype=np.float32)
    w_proj = np.asarray(w_proj, dtype=np.float32)
    b_proj = np.asarray(b_proj, dtype=np.float32)

    ki, qi = np.meshgrid(np.arange(P), np.arange(P), indexing="ij")
    causalT = np.where(ki > qi, np.float32(0.0), np.float32(1.0))
    # xT [NX, S] -> e-major [NE, P, S], split [:, :, :SC] / [:, :, SC:]
    x8 = [
        np.ascontiguousarray(x[b].T.astype(BF16).reshape(NE, P, S)) for b in range(B)
    ]
    bv_full = b_attn[2 * NX : 3 * NX].astype(np.float64)
    bp_eff = (b_proj.astype(np.float64) + bv_full @ w_proj.astype(np.float64)).astype(
        np.float32
    )
    bp_row32 = np.ascontiguousarray(bp_eff.reshape(1, NX))

    in_maps = []
    for c in range(8):
        b, g = divmod(c, 4)
        cols = slice(HDW * g, HDW * (g + 1))
        kcols = slice(NX + HDW * g, NX + HDW * (g + 1))
        vcols = slice(2 * NX + HDW * g, 2 * NX + HDW * (g + 1))
        bqk_arr = np.concatenate([b_attn[cols] * 0.125, b_attn[kcols]]).reshape(4, P)
        # fi-major q/k weights; the 1/8 score scale is folded into the q
        # columns (exact in bf16: pure exponent shift)
        wqk = np.concatenate(
            [w_attn[:, cols] * 0.125, w_attn[:, kcols]], axis=1
        )  # [NX, 512]
        wqkf = np.ascontiguousarray(
            wqk.reshape(NE, P, 4, P).transpose(2, 1, 0, 3).reshape(4, P, NX)
        ).astype(BF16)
        wvc = np.ascontiguousarray(
            w_attn[:, vcols].reshape(NE, P, HDW).transpose(1, 0, 2).reshape(P, NE * HDW)
        ).astype(BF16)
        # own-batch proj tiles (h, tp): rows 0:64 = in-batch sender 2tp's
        # head-h w_proj rows, 64:128 = sender 2tp+1's
        wtiles = np.zeros((HL, 2, P, NX), dtype=np.float32)
        for h in range(HL):
            for tp in range(2):
                for half, j in ((0, 2 * tp), (1, 2 * tp + 1)):
                    rows = w_proj[HDW * j + D * h : HDW * j + D * (h + 1), :]
                    wtiles[h, tp, 64 * half : 64 * half + D, :] = rows
        wpc = np.ascontiguousarray(
            wtiles.reshape(2 * HL, P, NX).transpose(1, 0, 2).reshape(P, 2 * HL * NX)
        ).astype(BF16)
        in_maps.append(
            {
                "x0": np.ascontiguousarray(x8[b][:, :, :SC]),
                "xr": np.ascontiguousarray(x8[b][:, :, SC:]),
                "wqkf": wqkf,
                "wvc": wvc,
                "wpc": wpc,
                "bqk": np.ascontiguousarray(bqk_arr),
                "bp32": bp_row32,
                "causb": causalT.astype(BF16),
                "onesb": np.ones((P, 4), dtype=BF16),
                "slotb": np.array(
                    [[1 - b, b, 0, 0]], dtype=np.int32
                ),  # [is_batch0, is_batch1]
            }
        )
    return in_maps


def assemble_out(results):
    out = np.empty((B, S, NX), dtype=np.float32)
    for c in range(8):
        b, g = divmod(c, 4)
        out[b, g * SC : (g + 1) * SC, :] = results[c]["out"]
    return out


def run(in_maps, trace=False):
    from concourse.bass_utils import run_bass_kernel_spmd

    nc = _get_compiled()
    return run_bass_kernel_spmd(nc, in_maps, core_ids=list(range(8)), trace=trace)


def kernel(**inputs) -> np.ndarray:
    in_maps = make_in_maps(**inputs)
    res = run(in_maps)
    return assemble_out(res.results)


if __name__ == "__main__":
    _get_compiled()
    print("build+compile OK")
